# revision 43
# baseline (speedup 1.0000x reference)
"""AttentionBlock (GroupNorm + 8-head self-attention + proj + residual) on 8 trn2 cores.

Sharding: data-parallel over batch B=16 -> 2 samples per core. No collectives.

Per-sample dataflow (C=512 channels, L=1024 pixels, 8 heads x 64 dims):
  - x (C, L) lives as 4 SBUF f32 tiles (128, 1024), channels on partitions; x
    stays resident until the proj residual add (no re-load).
  - GroupNorm: per-channel mean/var via bn_stats over L; 16-channel group
    aggregation + broadcast-back via tiny mask matmuls on the PE; rstd via a
    DVE-only rsqrt bit-hack (keeps the ACT engine exp-table resident, no
    table swaps).  h is written as fp8 channel-chunk-pair tiles (128,2,1024).
  - All four big GEMMs (qkv, v, attention AV, proj) run in fp8e4 DoubleRow
    perf mode - each matmul contracts TWO 128-row K-tiles at 0.5 cycles/row.
    Weights are pre-scaled x8 into fp8 (avoids subnormals), epilogues fold
    the /8 back in.  Only the S=K^T Q matmuls stay bf16 (their K=64
    contraction can't pair, and fp8 would add noise for no speed).
  - Attention per head pair, split by i-halves so PSUM double-buffers:
    S^T in bf16 (row-packed head pairs share the PE, K=64 each); exp on
    ScalarE with the 1/8 scale and a fixed -3 bias fused (cancels in the
    softmax ratio, keeps fp8 e < 240), writing fp8 e-pair tiles; AV
    DoubleRow-contracts both jc chunks of a pair, with the softmax
    denominator riding along as PSUM row 64.  Attention outputs are stored
    as raw/64 in fp8 (range safety); the denominator reciprocal (x64,
    reciprocal_approx_fast) is broadcast back per i-half via a K=2 selector
    matmul and one normalization multiply, emitted per half so the last
    pair's proj can start while its second half still runs.
  - proj + bias + residual, write out split across two DMA queues.
  - Cross-sample software pipeline: sample s+1's groupnorm/QKV/V fill the PE
    while ScalarE works through sample s's exps; sample s's proj fills the
    head of sample s+1's attention.

Startup: x and the six critical weight stages load on the two fast HWDGE
queues (sync/scalar) split in halves, with the gpsimd SWDGE queue taking the
second halves; first attention matmul starts ~13us in.
"""

import numpy as np

import concourse.bass as bass
import concourse.mybir as mybir
import concourse.tile as tile
from concourse import bacc
from concourse.bass_utils import run_bass_kernel_spmd
from concourse.masks import make_identity

F32 = mybir.dt.float32
F32R = mybir.dt.float32r
BF16 = mybir.dt.bfloat16
F8 = mybir.dt.float8e4
U32 = mybir.dt.uint32
AF = mybir.ActivationFunctionType
OP = mybir.AluOpType
PM = mybir.MatmulPerfMode

B, C, H, W = 16, 512, 32, 32
L = H * W
NH, HD = 8, 64
NG, GS = 32, 16
EPS = 1e-5
N_CORES = 8
BPC = B // N_CORES  # samples per core
P = 128
CK = C // P   # 4 channel chunks
LK = L // P   # 8 pixel chunks
SCALE = HD ** -0.5
ESHIFT = -3.0   # exp(x*scale + ESHIFT): cancels in softmax, keeps e < fp8 max
WSCALE = 8.0    # weights pre-scaled into fp8; epilogues multiply by 1/WSCALE
ADIV = 64.0     # attention outputs stored as raw/ADIV in fp8; rsum carries xADIV
RSQRT_MAGIC = 0x5F3759DF

_NC_CACHE = {}


class Ctx:
    pass


def _consts_early(nc, c, const, nw_d, nb_d, qb_d, pb_d):
    # bias vectors load as single-descriptor ROWS (a (128,1)-column DMA costs
    # ~1.4us of queue time; a contiguous row is free) - PE transposes turn
    # them into per-partition columns right after the identity exists
    c.nwrow = const.tile([1, C], F32, tag="nwrow")
    c.nbrow = const.tile([1, C], F32, tag="nbrow")
    c.pbrow = const.tile([1, C], F32, tag="pbrow")
    c.qbrow = const.tile([1, 3 * C], F32, tag="qbrow")
    nc.sync.dma_start(c.nwrow, nw_d.ap()[None, :])
    nc.sync.dma_start(c.nbrow, nb_d.ap()[None, :])
    nc.sync.dma_start(c.pbrow, pb_d.ap()[None, :])
    nc.sync.dma_start(c.qbrow, qb_d.ap()[None, :])

    # only what groupnorm + the first transposes need; everything else is
    # deferred so it can't block the DVE/gpsimd in-order queues at startup
    c.ident = const.tile([P, P], F32, tag="ident")
    make_identity(nc, c.ident)

    # gmask[kc][ch, g] = 1/16 iff global_channel // 16 == g   (128, 32)
    c.gmask = []
    for kc in range(CK):
        gm = const.tile([P, NG], F32, tag=f"gmask{kc}", name=f"gmask{kc}")
        nc.gpsimd.memset(gm, 1.0 / GS)
        nc.gpsimd.affine_select(
            out=gm, in_=gm, compare_op=OP.is_ge, fill=0.0,
            base=P * kc, channel_multiplier=1, pattern=[[-GS, NG]])
        nc.gpsimd.affine_select(
            out=gm, in_=gm, compare_op=OP.is_ge, fill=0.0,
            base=(GS - 1) - P * kc, channel_multiplier=-1, pattern=[[GS, NG]])
        c.gmask.append(gm)

    # bmask[g, ch] = 1 iff ch // 16 == g  (32, 512)
    c.bmask = const.tile([NG, C], F32, tag="bmask")
    nc.gpsimd.memset(c.bmask, 1.0)
    nc.gpsimd.affine_select(
        out=c.bmask, in_=c.bmask, compare_op=OP.is_ge, fill=0.0,
        base=0, channel_multiplier=-GS, pattern=[[1, C]])
    nc.gpsimd.affine_select(
        out=c.bmask, in_=c.bmask, compare_op=OP.is_ge, fill=0.0,
        base=GS - 1, channel_multiplier=GS, pattern=[[-1, C]])

    c.magic = const.tile([NG, 1], U32, tag="magic")
    nc.vector.memset(c.magic, RSQRT_MAGIC)
    c.eshift = const.tile([P, 1], F32, tag="eshift")
    nc.vector.memset(c.eshift, ESHIFT)
    c.ones1 = const.tile([1, P], F32, tag="ones1")
    nc.vector.memset(c.ones1, 1.0)


def _consts_late(nc, c, const):
    # sel2[h2, ch] = 1 iff ch // 64 == h2  (2, 128), f32r for full-rate matmul
    sel2s = const.tile([2, P], F32, tag="sel2s")
    nc.gpsimd.memset(sel2s, 1.0)
    nc.gpsimd.affine_select(
        out=sel2s, in_=sel2s, compare_op=OP.is_ge, fill=0.0,
        base=0, channel_multiplier=-HD, pattern=[[1, P]])
    nc.gpsimd.affine_select(
        out=sel2s, in_=sel2s, compare_op=OP.is_ge, fill=0.0,
        base=HD - 1, channel_multiplier=HD, pattern=[[-1, P]])
    c.sel2 = const.tile([2, P], F32R, tag="sel2")
    nc.vector.tensor_copy(out=c.sel2, in_=sel2s)


def _emit(nc, tc, pools, x_d, out_d, nw_d, nb_d, qw_d, qb_d, pw_d, pb_d):
    (const, stage, xp, hp_, qkp, vp, ep, attp, op_, sm, csp, ps, ps2) = pools

    qi = Ctx()
    qi.steady_i = 0

    def next_q():
        e = [nc.sync, nc.gpsimd][qi.steady_i % 2]
        qi.steady_i += 1
        return e

    c = Ctx()

    x_r = x_d.ap().rearrange("b (kc p) h w -> b kc p (h w)", p=P)
    o_r = out_d.ap().rearrange("b (kc p) h w -> b kc p (h w)", p=P)

    S = [Ctx() for _ in range(BPC)]
    for st_ in S:
        st_.x = [None] * CK

    def emit_x_load(s, engines, split=True):
        # split each (128, 1024) tile into two partition halves spread over
        # the queues so every kc chunk completes early
        st_ = S[s]
        for kc in range(CK):
            xt = xp.tile([P, L], F32, tag=f"x{kc}", name=f"x{kc}_{s}")
            if split:
                for half in range(2):
                    sl = slice(half * 64, (half + 1) * 64)
                    engines[(2 * kc + half) % len(engines)].dma_start(
                        xt[sl, :], x_r[s, kc][sl, :])
            else:
                engines[kc % len(engines)].dma_start(xt, x_r[s, kc])
            st_.x[kc] = xt

    def emit_bias_cols():
        # transpose the bias rows into per-partition columns in one PSUM
        # pass; broadcast the v bias via a K=1 ones-matmul
        bp = ps2.tile([P, 512], F32, tag="p2", name="bias_ps")
        one = c.ones1[0:1, 0:1]   # 1x1 identity for single-row transposes
        for kc in range(CK):
            nc.tensor.transpose(bp[:, kc:kc + 1],
                                c.nwrow[:, kc * P:(kc + 1) * P], one)
            nc.tensor.transpose(bp[:, 4 + kc:5 + kc],
                                c.nbrow[:, kc * P:(kc + 1) * P], one)
            nc.tensor.transpose(bp[:, 16 + kc:17 + kc],
                                c.pbrow[:, kc * P:(kc + 1) * P], one)
        for oc in range(8):
            nc.tensor.transpose(bp[:, 8 + oc:9 + oc],
                                c.qbrow[:, oc * P:(oc + 1) * P], one)
        bias_cols = const.tile([P, 20], F32, tag="bias_cols")
        nc.vector.tensor_copy(out=bias_cols, in_=bp[:, 0:20])
        c.nw = [bias_cols[:, kc:kc + 1] for kc in range(CK)]
        c.nb = [bias_cols[:, 4 + kc:5 + kc] for kc in range(CK)]
        c.qb = [bias_cols[:, 8 + oc:9 + oc] for oc in range(8)]
        c.pb = [bias_cols[:, 16 + kc:17 + kc] for kc in range(CK)]
        vps = ps2.tile([P, 512], F32, tag="p2", name="vb_ps")
        nc.tensor.matmul(vps, c.ones1, c.qbrow[:, 1024:1536],
                         start=True, stop=True)
        c.vb = const.tile([P, 512], F32, tag="vb")
        nc.vector.tensor_copy(out=c.vb, in_=vps)

    def emit_gn_stats(s):
        st_ = S[s]
        st_.stat2 = []
        for kc in range(CK):
            xt = st_.x[kc]
            bst = sm.tile([P, 2, 6], F32, tag="bst", name="bst")
            nc.vector.bn_stats(out=bst[:, 0, :], in_=xt[:, 0:512])
            nc.vector.bn_stats(out=bst[:, 1, :], in_=xt[:, 512:1024])
            mv = sm.tile([P, 2], F32, tag="mv", name="mv")
            nc.vector.bn_aggr(out=mv, in_=bst)
            st2 = sm.tile([P, 2], F32, tag="st2", name="st2")
            nc.vector.tensor_copy(out=st2[:, 0:1], in_=mv[:, 0:1])
            nc.vector.tensor_tensor(st2[:, 1:2], mv[:, 0:1], mv[:, 0:1], OP.mult)
            nc.vector.tensor_tensor(st2[:, 1:2], st2[:, 1:2], mv[:, 1:2], OP.add)
            st_.stat2.append(st2)

    def emit_rsqrt(dst, var):
        # dst = (var + EPS) ** -0.5 entirely on DVE: shift-subtract seed +
        # two Newton-Raphson passes (~1e-5 rel) - keeps ACT's exp table hot
        vv = sm.tile([NG, 1], F32, tag="vv", name="vv")
        y = sm.tile([NG, 1], F32, tag="y", name="y")
        t1 = sm.tile([NG, 1], F32, tag="t1", name="t1")
        nc.vector.tensor_scalar(vv, var, EPS, None, op0=OP.add)
        nc.vector.tensor_scalar(y.bitcast(U32), vv.bitcast(U32), 1, None,
                                op0=OP.logical_shift_right)
        nc.vector.tensor_tensor(y.bitcast(U32), c.magic, y.bitcast(U32),
                                OP.subtract)
        for _ in range(2):
            nc.vector.tensor_tensor(t1, vv, y, OP.mult)
            nc.vector.tensor_tensor(t1, t1, y, OP.mult)
            nc.vector.tensor_scalar(t1, t1, -0.5, 1.5, op0=OP.mult, op1=OP.add)
            nc.vector.tensor_tensor(y, y, t1, OP.mult)
        nc.vector.tensor_copy(out=dst, in_=y)

    def emit_gn_apply(s):
        st_ = S[s]
        gps = ps2.tile([P, 512], F32, tag="p2", name="gn_ps")
        for kc in range(CK):
            nc.tensor.matmul(gps[0:NG, 0:2], c.gmask[kc], st_.stat2[kc],
                             start=(kc == 0), stop=(kc == CK - 1))
        gst = sm.tile([NG, 2], F32, tag="gst", name="gst")
        gsb = sm.tile([NG, 2], F32, tag="gsb", name="gsb")
        gtmp = sm.tile([NG, 1], F32, tag="gtmp", name="gtmp")
        nc.vector.tensor_copy(out=gsb, in_=gps[0:NG, 0:2])
        nc.vector.tensor_tensor(gtmp, gsb[:, 0:1], gsb[:, 0:1], OP.mult)
        nc.vector.tensor_tensor(gtmp, gsb[:, 1:2], gtmp, OP.subtract)  # var
        emit_rsqrt(gst[:, 1:2], gtmp)                                  # rstd
        nc.vector.tensor_copy(out=gst[:, 0:1], in_=gsb[:, 0:1])       # gmean
        chps = ps2.tile([P, 512], F32, tag="p2", name="gn_ps2")
        for kc in range(CK):
            nc.tensor.matmul(chps[:, kc * 2: kc * 2 + 2],
                             c.bmask[:, kc * P:(kc + 1) * P], gst,
                             start=True, stop=True)
        st_.h = [None, None]   # fp8 channel-chunk-pair tiles (128, 2, 1024)
        for kc in range(CK):
            Acol = sm.tile([P, 1], F32, tag="Acol", name="Acol")
            Bcol = sm.tile([P, 1], F32, tag="Bcol", name="Bcol")
            nc.vector.tensor_tensor(Acol, chps[:, kc * 2 + 1: kc * 2 + 2],
                                    c.nw[kc], OP.mult)
            nc.vector.tensor_tensor(Bcol, chps[:, kc * 2: kc * 2 + 1], Acol, OP.mult)
            nc.vector.tensor_tensor(Bcol, c.nb[kc], Bcol, OP.subtract)
            kcp, t = kc // 2, kc % 2
            if st_.h[kcp] is None:
                st_.h[kcp] = hp_.tile([P, 2, L], F8, tag=f"h{kcp}",
                                      name=f"h{kcp}_{s}")
            # alternate DVE/GpSimd so the four h writes take ~2 writes of
            # wall time on the first-exp critical path
            eng = nc.vector if kc % 2 else nc.gpsimd
            with nc.allow_low_precision(reason="fp8 activations"):
                eng.tensor_scalar(st_.h[kcp][:, t, :], st_.x[kc],
                                  Acol, Bcol, op0=OP.mult, op1=OP.add)
        st_.qkT = [None] * 8
        st_.v = [None] * (LK // 2)
        st_.att = [None, None]  # fp8 pair tiles (128, 2, 1024), t = hp % 2

    qw_r4 = qw_d.ap().rearrange("(oc p) ch -> oc p ch", p=P)
    pw_r4 = pw_d.ap().rearrange("(oc p) ch -> oc p ch", p=P)
    # wT: (128, kcp, t, col) fp8, pre-scaled by WSCALE.  col < 1536 for qkv.
    c.wT = const.tile([P, 2, 2, 3 * C], F8, tag="wT", name="wT")
    c.pT = const.tile([P, 2, 2, C], F8, tag="pT", name="pT")

    def emit_tr_stage(oc, eng=None):
        src_r = qw_r4[oc] if oc < 12 else pw_r4[oc - 12]
        ws = stage.tile([P, C], F32, tag="wstage", name="wstage")
        (eng or next_q()).dma_start(ws, src_r)
        return ws

    def emit_tr_unit(oc, ws, on_act=False):
        col = (oc if oc < 12 else oc - 12) * P
        pt = ps2.tile([P, 512], F32, tag="p2", name="tr_ps")
        for kc in range(CK):
            nc.tensor.transpose(pt[:, kc * P:(kc + 1) * P],
                                ws[:, kc * P:(kc + 1) * P], c.ident)
        dst = c.wT if oc < 12 else c.pT
        dst_ap = dst[:, :, :, col:col + P]
        src_ap = pt.rearrange("p (a b i) -> p a b i", a=2, i=P)
        with nc.allow_low_precision(reason="fp8 weights, x8 prescaled"):
            if on_act:
                # startup only: ACT is idle pre-attention, and this keeps the
                # x8-cast copies out of the groupnorm-critical DVE queue
                nc.scalar.activation(dst_ap, src_ap, AF.Copy, scale=WSCALE)
            else:
                nc.vector.tensor_scalar(dst_ap, src_ap, WSCALE, None,
                                        op0=OP.mult)

    def emit_qkv_unit(s, oc, li):
        st_ = S[s]
        if st_.qkT[oc] is None:
            st_.qkT[oc] = qkp.tile([P, L], BF16, tag=f"qk{oc}", name=f"qk{oc}_{s}")
        dst = st_.qkT[oc]
        pt = ps2.tile([P, 512], F32, tag="p2", name="qkv_ps")
        for kcp in range(2):
            nc.tensor.matmul(pt,
                             c.wT[:, kcp, :, oc * P:(oc + 1) * P],
                             st_.h[kcp][:, :, li * 512:(li + 1) * 512],
                             start=(kcp == 0), stop=(kcp == 1),
                             perf_mode=PM.DoubleRow)
        nc.vector.tensor_scalar(dst[:, li * 512:(li + 1) * 512],
                                pt, 1.0 / WSCALE, c.qb[oc],
                                op0=OP.mult, op1=OP.add)

    def emit_qkv_qk(s, hp):
        for oc in (hp, 4 + hp):
            for li in range(2):
                emit_qkv_unit(s, oc, li)

    def emit_v(s, lcs):
        # v pair tiles: (128 jpix, 2 chunk-parity, 8 heads, 64+1+3pad) fp8;
        # head pitch 68 keeps DoubleRow ldweights panels 4-byte aligned
        st_ = S[s]
        for lc in lcs:
            lcp, t = lc // 2, lc % 2
            pt = ps2.tile([P, 512], F32, tag="p2", name="v_ps")
            for kcp in range(2):
                nc.tensor.matmul(pt,
                                 st_.h[kcp][:, :, lc * P:(lc + 1) * P],
                                 c.wT[:, kcp, :, 1024:1536],
                                 start=(kcp == 0), stop=(kcp == 1),
                                 perf_mode=PM.DoubleRow)
            if st_.v[lcp] is None:
                vt = vp.tile([P, 2, NH, HD + 4], F8, tag=f"v{lcp}",
                             name=f"v{lcp}_{s}")
                nc.vector.memset(vt[:, :, :, HD:HD + 1], 1.0)
                nc.vector.memset(vt[:, :, :, HD + 1:HD + 4], 0.0)
                st_.v[lcp] = vt
            vt = st_.v[lcp]
            with nc.allow_low_precision(reason="fp8 attention values"):
                nc.vector.scalar_tensor_tensor(
                    out=vt[:, t, :, 0:HD],
                    in0=pt.rearrange("p (h d) -> p h d", d=HD),
                    scalar=1.0 / WSCALE,
                    in1=c.vb.rearrange("p (h d) -> p h d", d=HD),
                    op0=OP.mult, op1=OP.add)

    fill_q = []    # units for the sample-0 attention window (drains first)
    fill_q2 = []   # reserve units held back for the sample-1 window

    def pop_fill(n=1):
        for _ in range(n):
            if fill_q:
                fill_q.pop(0)()
            elif fill_q2:
                fill_q2.pop(0)()

    def make_norm2(s, hp, li, rsum):
        st_ = S[s]

        def norm2():
            rb2 = ps2.tile([P, 512], F32, tag="p2", name="rb2_ps")
            nc.tensor.matmul(rb2, c.sel2, rsum[:, li * 512:(li + 1) * 512],
                             start=True, stop=True)
            kcp, t = hp // 2, hp % 2
            sl = st_.att[kcp][:, t, li * 512:(li + 1) * 512]
            with nc.allow_low_precision(reason="fp8 attention probs"):
                nc.vector.tensor_tensor(sl, sl, rb2, OP.mult)
        return norm2

    carry = Ctx()
    carry.stile = None

    def s_mms_for(s2, hp2, ic, jc):
        st2 = S[s2]
        kT, qT = st2.qkT[4 + hp2], st2.qkT[hp2]
        stile = ps.tile([P, 1024], F32, tag="s", name=f"s_{hp2}_{ic}_{jc}")
        for h2 in range(2):
            nc.tensor.matmul(
                stile[:, h2 * 512:(h2 + 1) * 512],
                kT[h2 * HD:(h2 + 1) * HD, jc * P:(jc + 1) * P],
                qT[h2 * HD:(h2 + 1) * HD, ic * 512:(ic + 1) * 512],
                start=True, stop=True)
        return stile

    def emit_pair(s, hp, tail_units=None):
        st_ = S[s]
        kcp_a, t_a = hp // 2, hp % 2
        if st_.att[kcp_a] is None:
            st_.att[kcp_a] = attp.tile([P, 2, L], F8, tag=f"att{kcp_a}",
                                       name=f"att{kcp_a}_{s}")
        csum = csp.tile([2, L], F32, tag="csum", name=f"csum_{s}_{hp}")

        def s_mms(ic, jc):
            return s_mms_for(s, hp, ic, jc)

        def next_block(ic):
            # the block whose first S-matmuls we prefetch during this
            # block's last step, so the exp stream never waits at a boundary
            if ic == 0:
                return (s, hp, 1)
            if hp < 3 and S[s].qkT[hp + 5] is not None:
                return (s, hp + 1, 0)
            return None

        for ic in range(2):
            av = ps.tile([P, 1024], F32, tag="s", name=f"av_{hp}_{ic}")
            if carry.stile is not None:
                stile, carry.stile = carry.stile, None
            else:
                stile = s_mms(ic, 0)
            for jcp in range(LK // 2):
                e_t = ep.tile([P, 2, 1024], F8, tag="e", name="e_t")
                for t in range(2):
                    jc = 2 * jcp + t
                    nc.scalar.activation(e_t[:, t, :], stile, AF.Exp,
                                         scale=SCALE, bias=c.eshift)
                    # emit next S ahead of this AV so the PE stream runs one
                    # step ahead of ScalarE; soak the PE with filler units
                    if jc + 1 < LK:
                        stile = s_mms(ic, jc + 1)
                    else:
                        nb = next_block(ic)
                        if nb is not None:
                            carry.stile = s_mms_for(nb[0], nb[1], nb[2], 0)
                    pop_fill()
                for h2 in range(2):
                    nc.tensor.matmul(
                        av[0:HD + 2, h2 * 512:(h2 + 1) * 512],
                        st_.v[jcp][:, :, 2 * hp + h2, 0:HD + 2],
                        e_t[:, :, h2 * 512:(h2 + 1) * 512],
                        start=(jcp == 0), stop=(jcp == LK // 2 - 1),
                        perf_mode=PM.DoubleRow)
            for h2 in range(2):
                with nc.allow_low_precision(reason="fp8, /64 range guard"):
                    nc.vector.tensor_scalar(
                        st_.att[kcp_a][h2 * HD:(h2 + 1) * HD, t_a,
                                       ic * 512:(ic + 1) * 512],
                        av[0:HD, h2 * 512:(h2 + 1) * 512],
                        1.0 / ADIV, None, op0=OP.mult)
                cstage = sm.tile([1, 512], F32, tag="cstage", name="cstage")
                nc.vector.tensor_copy(
                    out=cstage, in_=av[HD:HD + 1, h2 * 512:(h2 + 1) * 512])
                # the last pair's csum hop is tail-critical: use the scalar
                # queue (ACT is done with exps by then); sync is busy storing
                ceng = nc.scalar if (s == 1 and hp == 3) else nc.sync
                ceng.dma_start(csum[h2:h2 + 1, ic * 512:(ic + 1) * 512], cstage)
            # per-half denominators: approx reciprocal on DVE, x64 rounding
            # copy (f32r for the selector matmul) on gpsimd (DVE for the
            # tail-critical second sample), then the normalize unit queues
            rscr = csp.tile([2, 512], F32, tag="rscr", name=f"rscr_{s}_{hp}_{ic}")
            rsum = csp.tile([2, L], F32R, tag="rsum", name=f"rsum_{s}_{hp}",
                            bufs=2) if ic == 0 else st_.rsum_cur
            st_.rsum_cur = rsum
            with nc.allow_low_precision(reason="softmax denominators"):
                nc.vector.reciprocal_approx_fast(
                    out=rscr, in_=csum[:, ic * 512:(ic + 1) * 512])
            nc.vector.tensor_scalar(rsum[:, ic * 512:(ic + 1) * 512], rscr,
                                    ADIV, None, op0=OP.mult)
            fill_q.insert(min(len(fill_q), 6), make_norm2(s, hp, ic, rsum))
            if tail_units and tail_units.get(ic):
                pos = min(len(fill_q), 7)
                for u in tail_units[ic]:
                    fill_q.insert(pos, u)
                    pos += 1

    def emit_proj_unit(s, oc, li):
        st_ = S[s]
        pt = ps2.tile([P, 512], F32, tag="p2", name="proj_ps")
        for kcp in range(2):
            nc.tensor.matmul(pt,
                             c.pT[:, kcp, :, oc * P:(oc + 1) * P],
                             st_.att[kcp][:, :, li * 512:(li + 1) * 512],
                             start=(kcp == 0), stop=(kcp == 1),
                             perf_mode=PM.DoubleRow)
        ot = op_.tile([P, 512], F32, tag="ot", name="ot")
        nc.vector.tensor_scalar(ot, pt, 1.0 / WSCALE, c.pb[oc],
                                op0=OP.mult, op1=OP.add)
        nc.gpsimd.tensor_tensor(ot, ot, st_.x[oc][:, li * 512:(li + 1) * 512],
                                OP.add)
        for q in range(2):
            sl = slice(li * 512 + q * 256, li * 512 + (q + 1) * 256)
            eng = [nc.sync, nc.scalar][q] if s == 1 else next_q()
            eng.dma_start(o_r[s, oc][:, sl], ot[:, q * 256:(q + 1) * 256])

    # ---------------- schedule ----------------
    # startup: bias rows (single-descriptor) then x s0 on the two fast HWDGE
    # queues; critical weight stages interleave; gpsimd's SWDGE queue only
    # gets work after its engine finishes the mask consts
    _consts_early(nc, c, const, nw_d, nb_d, qb_d, pb_d)
    emit_x_load(0, [nc.sync, nc.scalar])
    ws_first = {}
    for oc, eng in ((0, nc.sync), (4, nc.scalar), (8, nc.gpsimd),
                    (9, nc.gpsimd), (10, nc.sync), (11, nc.scalar)):
        ws_first[oc] = emit_tr_stage(oc, eng)
    emit_x_load(1, [nc.sync, nc.scalar])
    emit_bias_cols()
    emit_gn_stats(0)
    for oc in (0, 4):
        emit_tr_unit(oc, ws_first[oc])
    emit_gn_apply(0)
    emit_qkv_qk(0, 0)         # pair(0,0) q/k: its DVE epilogues gate the
    for oc in (8, 9, 10, 11):
        emit_tr_unit(oc, ws_first[oc])
    _consts_late(nc, c, const)
    emit_v(0, [0, 1, 2, 3])   # first S-matmuls, so they go before gn(1)
    emit_gn_stats(1)
    emit_gn_apply(1)

    # everything else becomes filler units popped per attention step; the
    # fill_q order encodes the just-in-time deadlines.  Units that may wait
    # until the sample-1 attention window go to the fill_q2 reserve, so the
    # PE stays fed (and at high p-state) through BOTH windows.
    for lc in range(4, LK):
        fill_q.append(lambda lc=lc: emit_v(0, [lc]))
    for oc_t, oc_a, oc_b in ((1, 1, 5), (2, 2, 6), (3, 3, 7)):
        fill_q.append(lambda oc=oc_t: emit_tr_unit(oc, emit_tr_stage(oc)))
        fill_q.append(lambda oc=oc_t: emit_tr_unit(oc + 4, emit_tr_stage(oc + 4)))
        for li in range(2):
            fill_q.append(lambda oc=oc_a, li=li: emit_qkv_unit(0, oc, li))
        for li in range(2):
            fill_q.append(lambda oc=oc_b, li=li: emit_qkv_unit(0, oc, li))
    for oc in (12, 13, 14, 15):       # proj weights, needed from pair(1,0)
        fill_q.append(lambda oc=oc: emit_tr_unit(oc, emit_tr_stage(oc)))
    for oc in (0, 4):                 # pair(1,0)'s q/k - before its S mms
        for li in range(2):
            fill_q.append(lambda oc=oc, li=li: emit_qkv_unit(1, oc, li))
    for lc in range(LK):              # all v(1) - consumed from pair(1,0) on
        fill_q.append(lambda lc=lc: emit_v(1, [lc]))
    for oc in (1, 5, 2, 6, 3, 7):     # later pairs' q/k ride the reserve
        for li in range(2):
            fill_q2.append(lambda oc=oc, li=li: emit_qkv_unit(1, oc, li))

    for hp in range(3):
        emit_pair(0, hp)
    emit_pair(0, 3, tail_units={
        0: [lambda oc=oc: emit_proj_unit(0, oc, 0) for oc in range(CK)],
        1: [lambda oc=oc: emit_proj_unit(0, oc, 1) for oc in range(CK)],
    })
    while fill_q:   # anything pair(1,0) needs that hasn't popped yet
        pop_fill()

    for hp in range(3):
        emit_pair(1, hp)
    emit_pair(1, 3, tail_units={
        0: [lambda oc=oc: emit_proj_unit(1, oc, 0) for oc in range(CK)],
        1: [lambda oc=oc: emit_proj_unit(1, oc, 1) for oc in range(CK)],
    })
    while fill_q or fill_q2:
        pop_fill()


def _build():
    if "nc" in _NC_CACHE:
        return _NC_CACHE["nc"]
    nc = bacc.Bacc("TRN2", target_bir_lowering=False, debug=False)
    x_d = nc.dram_tensor("x", (BPC, C, H, W), F32, kind="ExternalInput")
    nw_d = nc.dram_tensor("norm_w", (C,), F32, kind="ExternalInput")
    nb_d = nc.dram_tensor("norm_b", (C,), F32, kind="ExternalInput")
    qw_d = nc.dram_tensor("qkv_w", (3 * C, C), F32, kind="ExternalInput")
    qb_d = nc.dram_tensor("qkv_b", (3 * C,), F32, kind="ExternalInput")
    pw_d = nc.dram_tensor("proj_w", (C, C), F32, kind="ExternalInput")
    pb_d = nc.dram_tensor("proj_b", (C,), F32, kind="ExternalInput")
    out_d = nc.dram_tensor("out", (BPC, C, H, W), F32, kind="ExternalOutput")
    with tile.TileContext(nc) as tc:
        with (
            tc.tile_pool(name="const", bufs=1) as const,
            tc.tile_pool(name="stage", bufs=6) as stage,
            tc.tile_pool(name="xp", bufs=2) as xp,
            tc.tile_pool(name="hp", bufs=2) as hp_,
            tc.tile_pool(name="qkp", bufs=2) as qkp,
            tc.tile_pool(name="vp", bufs=2) as vp,
            tc.tile_pool(name="ep", bufs=3) as ep,
            tc.tile_pool(name="attp", bufs=2) as attp,
            tc.tile_pool(name="op", bufs=2) as op_,
            tc.tile_pool(name="sm", bufs=1) as sm,
            tc.tile_pool(name="csp", bufs=2) as csp,
            tc.tile_pool(name="ps", bufs=3, space="PSUM") as ps,
            tc.tile_pool(name="ps2", bufs=2, space="PSUM") as ps2,
        ):
            pools = (const, stage, xp, hp_, qkp, vp, ep, attp, op_, sm, csp, ps, ps2)
            _emit(nc, tc, pools, x_d, out_d, nw_d, nb_d, qw_d, qb_d, pw_d, pb_d)
    nc.compile()
    _NC_CACHE["nc"] = nc
    return nc


def kernel(x, norm_w, norm_b, qkv_w, qkv_b, proj_w, proj_b):
    x = np.ascontiguousarray(x, dtype=np.float32)
    args = {
        "norm_w": np.ascontiguousarray(norm_w, np.float32),
        "norm_b": np.ascontiguousarray(norm_b, np.float32),
        "qkv_w": np.ascontiguousarray(qkv_w, np.float32),
        "qkv_b": np.ascontiguousarray(qkv_b, np.float32),
        "proj_w": np.ascontiguousarray(proj_w, np.float32),
        "proj_b": np.ascontiguousarray(proj_b, np.float32),
    }
    nc = _build()
    in_maps = [dict(args, x=x[i * BPC:(i + 1) * BPC]) for i in range(N_CORES)]
    res = run_bass_kernel_spmd(nc, in_maps, core_ids=list(range(N_CORES)))
    return np.concatenate([r["out"] for r in res.results], axis=0)


# revision 45
# speedup vs baseline: 1.1909x; 1.1909x over previous
"""AttentionBlock (GroupNorm + 8-head self-attention + proj + residual) on 8 trn2 cores.

Sharding: data-parallel over batch B=16 -> 2 samples per core. No collectives.

Per-sample dataflow (C=512 channels, L=1024 pixels, 8 heads x 64 dims):
  - x (C, L) lives as 4 SBUF f32 tiles (128, 1024), channels on partitions; x
    stays resident until the proj residual add (no re-load).
  - GroupNorm: per-channel mean/var via bn_stats over L; 16-channel group
    aggregation + broadcast-back via tiny mask matmuls on the PE; rstd via a
    DVE-only rsqrt bit-hack (keeps the ACT engine exp-table resident, no
    table swaps).  h is written as fp8 channel-chunk-pair tiles (128,2,1024).
  - All four big GEMMs (qkv, v, attention AV, proj) run in fp8e4 DoubleRow
    perf mode - each matmul contracts TWO 128-row K-tiles at 0.5 cycles/row.
    Weights are pre-scaled x8 into fp8 (avoids subnormals), epilogues fold
    the /8 back in.  Only the S=K^T Q matmuls stay bf16 (their K=64
    contraction can't pair, and fp8 would add noise for no speed).
  - Attention per head pair, split by i-halves so PSUM double-buffers:
    S^T in bf16 (row-packed head pairs share the PE, K=64 each); exp on
    ScalarE with the 1/8 scale and a fixed -3 bias fused (cancels in the
    softmax ratio, keeps fp8 e < 240), writing fp8 e-pair tiles; AV
    DoubleRow-contracts both jc chunks of a pair, with the softmax
    denominator riding along as PSUM row 64.  Attention outputs are stored
    as raw/64 in fp8 (range safety); the denominator reciprocal (x64,
    reciprocal_approx_fast) is broadcast back per i-half via a K=2 selector
    matmul and one normalization multiply, emitted per half so the last
    pair's proj can start while its second half still runs.
  - proj + bias + residual, write out split across two DMA queues.
  - Cross-sample software pipeline: sample s+1's groupnorm/QKV/V fill the PE
    while ScalarE works through sample s's exps; sample s's proj fills the
    head of sample s+1's attention.

Startup: x and the six critical weight stages load on the two fast HWDGE
queues (sync/scalar) split in halves, with the gpsimd SWDGE queue taking the
second halves; first attention matmul starts ~13us in.
"""

import numpy as np

import concourse.bass as bass
import concourse.mybir as mybir
import concourse.tile as tile
from concourse import bacc
from concourse.bass_utils import run_bass_kernel_spmd
from concourse.masks import make_identity

F32 = mybir.dt.float32
F32R = mybir.dt.float32r
BF16 = mybir.dt.bfloat16
F8 = mybir.dt.float8e4
U32 = mybir.dt.uint32
AF = mybir.ActivationFunctionType
OP = mybir.AluOpType
PM = mybir.MatmulPerfMode

B, C, H, W = 16, 512, 32, 32
L = H * W
NH, HD = 8, 64
NG, GS = 32, 16
EPS = 1e-5
N_CORES = 8
BPC = B // N_CORES  # samples per core
P = 128
CK = C // P   # 4 channel chunks
LK = L // P   # 8 pixel chunks
SCALE = HD ** -0.5
ESHIFT = -3.0   # exp(x*scale + ESHIFT): cancels in softmax, keeps e < fp8 max
WSCALE = 8.0    # weights pre-scaled into fp8; epilogues multiply by 1/WSCALE
ADIV = 64.0     # attention outputs stored as raw/ADIV in fp8; rsum carries xADIV
RSQRT_MAGIC = 0x5F3759DF

_NC_CACHE = {}


class Ctx:
    pass


def _consts_early(nc, c, const, nw_d, nb_d, qb_d, pb_d):
    # bias vectors load as single-descriptor ROWS (a (128,1)-column DMA costs
    # ~1.4us of queue time; a contiguous row is free) - PE transposes turn
    # them into per-partition columns right after the identity exists
    c.nwrow = const.tile([1, C], F32, tag="nwrow")
    c.nbrow = const.tile([1, C], F32, tag="nbrow")
    c.pbrow = const.tile([1, C], F32, tag="pbrow")
    c.qbrow = const.tile([1, 3 * C], F32, tag="qbrow")
    nc.sync.dma_start(c.nwrow, nw_d.ap()[None, :])
    nc.sync.dma_start(c.nbrow, nb_d.ap()[None, :])
    nc.sync.dma_start(c.pbrow, pb_d.ap()[None, :])
    nc.sync.dma_start(c.qbrow, qb_d.ap()[None, :])

    # only what groupnorm + the first transposes need; everything else is
    # deferred so it can't block the DVE/gpsimd in-order queues at startup
    c.ident = const.tile([P, P], F32, tag="ident")
    make_identity(nc, c.ident)

    # gmask[kc][ch, g] = 1/16 iff global_channel // 16 == g   (128, 32)
    c.gmask = []
    for kc in range(CK):
        gm = const.tile([P, NG], F32, tag=f"gmask{kc}", name=f"gmask{kc}")
        nc.gpsimd.memset(gm, 1.0 / GS)
        nc.gpsimd.affine_select(
            out=gm, in_=gm, compare_op=OP.is_ge, fill=0.0,
            base=P * kc, channel_multiplier=1, pattern=[[-GS, NG]])
        nc.gpsimd.affine_select(
            out=gm, in_=gm, compare_op=OP.is_ge, fill=0.0,
            base=(GS - 1) - P * kc, channel_multiplier=-1, pattern=[[GS, NG]])
        c.gmask.append(gm)

    # bmask[g, ch] = 1 iff ch // 16 == g  (32, 512)
    c.bmask = const.tile([NG, C], F32, tag="bmask")
    nc.gpsimd.memset(c.bmask, 1.0)
    nc.gpsimd.affine_select(
        out=c.bmask, in_=c.bmask, compare_op=OP.is_ge, fill=0.0,
        base=0, channel_multiplier=-GS, pattern=[[1, C]])
    nc.gpsimd.affine_select(
        out=c.bmask, in_=c.bmask, compare_op=OP.is_ge, fill=0.0,
        base=GS - 1, channel_multiplier=GS, pattern=[[-1, C]])

    c.magic = const.tile([NG, 1], U32, tag="magic")
    nc.vector.memset(c.magic, RSQRT_MAGIC)
    c.eshift = const.tile([P, 1], F32, tag="eshift")
    nc.vector.memset(c.eshift, ESHIFT)
    c.ones1 = const.tile([1, P], F32, tag="ones1")
    nc.vector.memset(c.ones1, 1.0)


def _consts_late(nc, c, const):
    # sel2[h2, ch] = 1 iff ch // 64 == h2  (2, 128), f32r for full-rate matmul
    sel2s = const.tile([2, P], F32, tag="sel2s")
    nc.gpsimd.memset(sel2s, 1.0)
    nc.gpsimd.affine_select(
        out=sel2s, in_=sel2s, compare_op=OP.is_ge, fill=0.0,
        base=0, channel_multiplier=-HD, pattern=[[1, P]])
    nc.gpsimd.affine_select(
        out=sel2s, in_=sel2s, compare_op=OP.is_ge, fill=0.0,
        base=HD - 1, channel_multiplier=HD, pattern=[[-1, P]])
    c.sel2 = const.tile([2, P], F32R, tag="sel2")
    nc.vector.tensor_copy(out=c.sel2, in_=sel2s)


def _emit(nc, tc, pools, x_d, out_d, nw_d, nb_d, qw_d, qb_d, pw_d, pb_d):
    (const, stage, xp, hp_, qkp, vp, ep, attp, op_, sm, csp, ps, ps2) = pools

    qi = Ctx()
    qi.steady_i = 0

    def next_q():
        e = [nc.sync, nc.gpsimd][qi.steady_i % 2]
        qi.steady_i += 1
        return e

    c = Ctx()

    x_r = x_d.ap().rearrange("b (kc p) h w -> b kc p (h w)", p=P)
    o_r = out_d.ap().rearrange("b (kc p) h w -> b kc p (h w)", p=P)

    S = [Ctx() for _ in range(BPC)]
    for st_ in S:
        st_.x = [None] * CK

    def emit_x_load(s, engines, split=True):
        # split each (128, 1024) tile into two partition halves spread over
        # the queues so every kc chunk completes early
        st_ = S[s]
        for kc in range(CK):
            xt = xp.tile([P, L], F32, tag=f"x{kc}", name=f"x{kc}_{s}")
            if split:
                for half in range(2):
                    sl = slice(half * 64, (half + 1) * 64)
                    engines[(2 * kc + half) % len(engines)].dma_start(
                        xt[sl, :], x_r[s, kc][sl, :])
            else:
                engines[kc % len(engines)].dma_start(xt, x_r[s, kc])
            st_.x[kc] = xt

    def emit_bias_cols():
        # transpose the bias rows into per-partition columns in one PSUM
        # pass; broadcast the v bias via a K=1 ones-matmul
        bp = ps2.tile([P, 512], F32, tag="p2", name="bias_ps")
        one = c.ones1[0:1, 0:1]   # 1x1 identity for single-row transposes
        for kc in range(CK):
            nc.tensor.transpose(bp[:, kc:kc + 1],
                                c.nwrow[:, kc * P:(kc + 1) * P], one)
            nc.tensor.transpose(bp[:, 4 + kc:5 + kc],
                                c.nbrow[:, kc * P:(kc + 1) * P], one)
            nc.tensor.transpose(bp[:, 16 + kc:17 + kc],
                                c.pbrow[:, kc * P:(kc + 1) * P], one)
        for oc in range(8):
            nc.tensor.transpose(bp[:, 8 + oc:9 + oc],
                                c.qbrow[:, oc * P:(oc + 1) * P], one)
        bias_cols = const.tile([P, 20], F32, tag="bias_cols")
        nc.vector.tensor_copy(out=bias_cols, in_=bp[:, 0:20])
        c.nw = [bias_cols[:, kc:kc + 1] for kc in range(CK)]
        c.nb = [bias_cols[:, 4 + kc:5 + kc] for kc in range(CK)]
        c.qb = [bias_cols[:, 8 + oc:9 + oc] for oc in range(8)]
        c.pb = [bias_cols[:, 16 + kc:17 + kc] for kc in range(CK)]
        vps = ps2.tile([P, 512], F32, tag="p2", name="vb_ps")
        nc.tensor.matmul(vps, c.ones1, c.qbrow[:, 1024:1536],
                         start=True, stop=True)
        c.vb = const.tile([P, 512], F32, tag="vb")
        nc.vector.tensor_copy(out=c.vb, in_=vps)

    def emit_gn_stats(s):
        st_ = S[s]
        st_.stat2 = []
        for kc in range(CK):
            xt = st_.x[kc]
            bst = sm.tile([P, 2, 6], F32, tag="bst", name="bst")
            nc.vector.bn_stats(out=bst[:, 0, :], in_=xt[:, 0:512])
            nc.vector.bn_stats(out=bst[:, 1, :], in_=xt[:, 512:1024])
            mv = sm.tile([P, 2], F32, tag="mv", name="mv")
            nc.vector.bn_aggr(out=mv, in_=bst)
            st2 = sm.tile([P, 2], F32, tag="st2", name="st2")
            nc.vector.tensor_copy(out=st2[:, 0:1], in_=mv[:, 0:1])
            nc.vector.tensor_tensor(st2[:, 1:2], mv[:, 0:1], mv[:, 0:1], OP.mult)
            nc.vector.tensor_tensor(st2[:, 1:2], st2[:, 1:2], mv[:, 1:2], OP.add)
            st_.stat2.append(st2)

    def emit_rsqrt(dst, var):
        # dst = (var + EPS) ** -0.5 entirely on DVE: shift-subtract seed +
        # two Newton-Raphson passes (~1e-5 rel) - keeps ACT's exp table hot
        vv = sm.tile([NG, 1], F32, tag="vv", name="vv")
        y = sm.tile([NG, 1], F32, tag="y", name="y")
        t1 = sm.tile([NG, 1], F32, tag="t1", name="t1")
        nc.vector.tensor_scalar(vv, var, EPS, None, op0=OP.add)
        nc.vector.tensor_scalar(y.bitcast(U32), vv.bitcast(U32), 1, None,
                                op0=OP.logical_shift_right)
        nc.vector.tensor_tensor(y.bitcast(U32), c.magic, y.bitcast(U32),
                                OP.subtract)
        for _ in range(2):
            nc.vector.tensor_tensor(t1, vv, y, OP.mult)
            nc.vector.tensor_tensor(t1, t1, y, OP.mult)
            nc.vector.tensor_scalar(t1, t1, -0.5, 1.5, op0=OP.mult, op1=OP.add)
            nc.vector.tensor_tensor(y, y, t1, OP.mult)
        nc.vector.tensor_copy(out=dst, in_=y)

    def emit_gn_apply(s):
        st_ = S[s]
        gps = ps2.tile([P, 512], F32, tag="p2", name="gn_ps")
        for kc in range(CK):
            nc.tensor.matmul(gps[0:NG, 0:2], c.gmask[kc], st_.stat2[kc],
                             start=(kc == 0), stop=(kc == CK - 1))
        gst = sm.tile([NG, 2], F32, tag="gst", name="gst")
        gsb = sm.tile([NG, 2], F32, tag="gsb", name="gsb")
        gtmp = sm.tile([NG, 1], F32, tag="gtmp", name="gtmp")
        nc.vector.tensor_copy(out=gsb, in_=gps[0:NG, 0:2])
        nc.vector.tensor_tensor(gtmp, gsb[:, 0:1], gsb[:, 0:1], OP.mult)
        nc.vector.tensor_tensor(gtmp, gsb[:, 1:2], gtmp, OP.subtract)  # var
        emit_rsqrt(gst[:, 1:2], gtmp)                                  # rstd
        nc.vector.tensor_copy(out=gst[:, 0:1], in_=gsb[:, 0:1])       # gmean
        chps = ps2.tile([P, 512], F32, tag="p2", name="gn_ps2")
        for kc in range(CK):
            nc.tensor.matmul(chps[:, kc * 2: kc * 2 + 2],
                             c.bmask[:, kc * P:(kc + 1) * P], gst,
                             start=True, stop=True)
        st_.h = [None, None]   # fp8 channel-chunk-pair tiles (128, 2, 1024)
        for kc in range(CK):
            Acol = sm.tile([P, 1], F32, tag="Acol", name="Acol")
            Bcol = sm.tile([P, 1], F32, tag="Bcol", name="Bcol")
            nc.vector.tensor_tensor(Acol, chps[:, kc * 2 + 1: kc * 2 + 2],
                                    c.nw[kc], OP.mult)
            nc.vector.tensor_tensor(Bcol, chps[:, kc * 2: kc * 2 + 1], Acol, OP.mult)
            nc.vector.tensor_tensor(Bcol, c.nb[kc], Bcol, OP.subtract)
            kcp, t = kc // 2, kc % 2
            if st_.h[kcp] is None:
                st_.h[kcp] = hp_.tile([P, 2, L], F8, tag=f"h{kcp}",
                                      name=f"h{kcp}_{s}")
            # alternate DVE/GpSimd so the four h writes take ~2 writes of
            # wall time on the first-exp critical path
            eng = nc.vector if kc % 2 else nc.gpsimd
            with nc.allow_low_precision(reason="fp8 activations"):
                eng.tensor_scalar(st_.h[kcp][:, t, :], st_.x[kc],
                                  Acol, Bcol, op0=OP.mult, op1=OP.add)
        st_.qkT = [None] * 8
        st_.v = [None] * (LK // 2)
        st_.att = [None, None]  # fp8 pair tiles (128, 2, 1024), t = hp % 2

    qw_r4 = qw_d.ap().rearrange("(oc p) ch -> oc p ch", p=P)
    pw_r4 = pw_d.ap().rearrange("(oc p) ch -> oc p ch", p=P)
    # wT: (128, kcp, t, col) fp8, pre-scaled by WSCALE.  col < 1536 for qkv.
    c.wT = const.tile([P, 2, 2, 3 * C], F8, tag="wT", name="wT")
    c.pT = const.tile([P, 2, 2, C], F8, tag="pT", name="pT")

    def emit_tr_stage(oc, eng=None):
        src_r = qw_r4[oc] if oc < 12 else pw_r4[oc - 12]
        ws = stage.tile([P, C], F32, tag="wstage", name="wstage")
        (eng or next_q()).dma_start(ws, src_r)
        return ws

    def emit_tr_unit(oc, ws, on_act=False):
        col = (oc if oc < 12 else oc - 12) * P
        pt = ps2.tile([P, 512], F32, tag="p2", name="tr_ps")
        for kc in range(CK):
            nc.tensor.transpose(pt[:, kc * P:(kc + 1) * P],
                                ws[:, kc * P:(kc + 1) * P], c.ident)
        dst = c.wT if oc < 12 else c.pT
        dst_ap = dst[:, :, :, col:col + P]
        src_ap = pt.rearrange("p (a b i) -> p a b i", a=2, i=P)
        with nc.allow_low_precision(reason="fp8 weights, x8 prescaled"):
            if on_act:
                # startup only: ACT is idle pre-attention, and this keeps the
                # x8-cast copies out of the groupnorm-critical DVE queue
                nc.scalar.activation(dst_ap, src_ap, AF.Copy, scale=WSCALE)
            else:
                nc.vector.tensor_scalar(dst_ap, src_ap, WSCALE, None,
                                        op0=OP.mult)

    def emit_qkv_unit(s, oc, li):
        st_ = S[s]
        if st_.qkT[oc] is None:
            st_.qkT[oc] = qkp.tile([P, L], BF16, tag=f"qk{oc}", name=f"qk{oc}_{s}")
        dst = st_.qkT[oc]
        pt = ps2.tile([P, 512], F32, tag="p2", name="qkv_ps")
        for kcp in range(2):
            nc.tensor.matmul(pt,
                             c.wT[:, kcp, :, oc * P:(oc + 1) * P],
                             st_.h[kcp][:, :, li * 512:(li + 1) * 512],
                             start=(kcp == 0), stop=(kcp == 1),
                             perf_mode=PM.DoubleRow)
        nc.vector.tensor_scalar(dst[:, li * 512:(li + 1) * 512],
                                pt, 1.0 / WSCALE, c.qb[oc],
                                op0=OP.mult, op1=OP.add)

    def emit_qkv_qk(s, hp):
        for oc in (hp, 4 + hp):
            for li in range(2):
                emit_qkv_unit(s, oc, li)

    def emit_v(s, lcs):
        # v pair tiles: (128 jpix, 2 chunk-parity, 8 heads, 64+1+3pad) fp8;
        # head pitch 68 keeps DoubleRow ldweights panels 4-byte aligned
        st_ = S[s]
        for lc in lcs:
            lcp, t = lc // 2, lc % 2
            pt = ps2.tile([P, 512], F32, tag="p2", name="v_ps")
            for kcp in range(2):
                nc.tensor.matmul(pt,
                                 st_.h[kcp][:, :, lc * P:(lc + 1) * P],
                                 c.wT[:, kcp, :, 1024:1536],
                                 start=(kcp == 0), stop=(kcp == 1),
                                 perf_mode=PM.DoubleRow)
            if st_.v[lcp] is None:
                vt = vp.tile([P, 2, NH, HD + 4], F8, tag=f"v{lcp}",
                             name=f"v{lcp}_{s}")
                nc.vector.memset(vt[:, :, :, HD:HD + 1], 1.0)
                nc.vector.memset(vt[:, :, :, HD + 1:HD + 4], 0.0)
                st_.v[lcp] = vt
            vt = st_.v[lcp]
            with nc.allow_low_precision(reason="fp8 attention values"):
                nc.vector.scalar_tensor_tensor(
                    out=vt[:, t, :, 0:HD],
                    in0=pt.rearrange("p (h d) -> p h d", d=HD),
                    scalar=1.0 / WSCALE,
                    in1=c.vb.rearrange("p (h d) -> p h d", d=HD),
                    op0=OP.mult, op1=OP.add)

    fill_q = []    # units for the sample-0 attention window (drains first)
    fill_q2 = []   # reserve units held back for the sample-1 window

    def pop_fill(n=1):
        for _ in range(n):
            if fill_q:
                fill_q.pop(0)()
            elif fill_q2:
                fill_q2.pop(0)()

    def make_norm2(s, hp, li, rsum):
        st_ = S[s]

        def norm2():
            rb2 = ps2.tile([P, 512], F32, tag="p2", name="rb2_ps")
            nc.tensor.matmul(rb2, c.sel2, rsum[:, li * 512:(li + 1) * 512],
                             start=True, stop=True)
            kcp, t = hp // 2, hp % 2
            sl = st_.att[kcp][:, t, li * 512:(li + 1) * 512]
            with nc.allow_low_precision(reason="fp8 attention probs"):
                nc.vector.tensor_tensor(sl, sl, rb2, OP.mult)
        return norm2

    carry = Ctx()
    carry.stile = None

    def s_mms_for(s2, hp2, ic, jc):
        st2 = S[s2]
        kT, qT = st2.qkT[4 + hp2], st2.qkT[hp2]
        stile = ps.tile([P, 1024], F32, tag="s", name=f"s_{hp2}_{ic}_{jc}")
        for h2 in range(2):
            nc.tensor.matmul(
                stile[:, h2 * 512:(h2 + 1) * 512],
                kT[h2 * HD:(h2 + 1) * HD, jc * P:(jc + 1) * P],
                qT[h2 * HD:(h2 + 1) * HD, ic * 512:(ic + 1) * 512],
                start=True, stop=True)
        return stile

    def emit_pair(s, hp, tail_units=None):
        st_ = S[s]
        kcp_a, t_a = hp // 2, hp % 2
        if st_.att[kcp_a] is None:
            st_.att[kcp_a] = attp.tile([P, 2, L], F8, tag=f"att{kcp_a}",
                                       name=f"att{kcp_a}_{s}")
        csum = csp.tile([2, L], F32, tag="csum", name=f"csum_{s}_{hp}")

        def s_mms(ic, jc):
            return s_mms_for(s, hp, ic, jc)

        def next_block(ic):
            # the block whose first S-matmuls we prefetch during this
            # block's last step, so the exp stream never waits at a boundary
            if ic == 0:
                return (s, hp, 1)
            if hp < 3 and S[s].qkT[hp + 5] is not None:
                return (s, hp + 1, 0)
            return None

        for ic in range(2):
            av = ps.tile([P, 1024], F32, tag="s", name=f"av_{hp}_{ic}")
            if carry.stile is not None:
                stile, carry.stile = carry.stile, None
            else:
                stile = s_mms(ic, 0)
            for jcp in range(LK // 2):
                e_t = ep.tile([P, 2, 1024], F8, tag="e", name="e_t")
                for t in range(2):
                    jc = 2 * jcp + t
                    nc.scalar.activation(e_t[:, t, :], stile, AF.Exp,
                                         scale=SCALE, bias=c.eshift)
                    # emit next S ahead of this AV so the PE stream runs one
                    # step ahead of ScalarE; soak the PE with filler units
                    if jc + 1 < LK:
                        stile = s_mms(ic, jc + 1)
                    else:
                        nb = next_block(ic)
                        if nb is not None:
                            carry.stile = s_mms_for(nb[0], nb[1], nb[2], 0)
                    pop_fill()
                for h2 in range(2):
                    nc.tensor.matmul(
                        av[0:HD + 2, h2 * 512:(h2 + 1) * 512],
                        st_.v[jcp][:, :, 2 * hp + h2, 0:HD + 2],
                        e_t[:, :, h2 * 512:(h2 + 1) * 512],
                        start=(jcp == 0), stop=(jcp == LK // 2 - 1),
                        perf_mode=PM.DoubleRow)
            for h2 in range(2):
                with nc.allow_low_precision(reason="fp8, /64 range guard"):
                    nc.vector.tensor_scalar(
                        st_.att[kcp_a][h2 * HD:(h2 + 1) * HD, t_a,
                                       ic * 512:(ic + 1) * 512],
                        av[0:HD, h2 * 512:(h2 + 1) * 512],
                        1.0 / ADIV, None, op0=OP.mult)
                cstage = sm.tile([1, 512], F32, tag="cstage", name="cstage")
                nc.vector.tensor_copy(
                    out=cstage, in_=av[HD:HD + 1, h2 * 512:(h2 + 1) * 512])
                # NOTE: never issue DMAs from the scalar engine - any HWDGE
                # work on ACT slows every activation ~20% for the whole run
                nc.sync.dma_start(csum[h2:h2 + 1, ic * 512:(ic + 1) * 512], cstage)
            # per-half denominators: approx reciprocal on DVE, x64 rounding
            # copy (f32r for the selector matmul) on gpsimd (DVE for the
            # tail-critical second sample), then the normalize unit queues
            rscr = csp.tile([2, 512], F32, tag="rscr", name=f"rscr_{s}_{hp}_{ic}")
            rsum = csp.tile([2, L], F32R, tag="rsum", name=f"rsum_{s}_{hp}",
                            bufs=2) if ic == 0 else st_.rsum_cur
            st_.rsum_cur = rsum
            with nc.allow_low_precision(reason="softmax denominators"):
                nc.vector.reciprocal_approx_fast(
                    out=rscr, in_=csum[:, ic * 512:(ic + 1) * 512])
            nc.vector.tensor_scalar(rsum[:, ic * 512:(ic + 1) * 512], rscr,
                                    ADIV, None, op0=OP.mult)
            fill_q.insert(min(len(fill_q), 6), make_norm2(s, hp, ic, rsum))
            if tail_units and tail_units.get(ic):
                pos = min(len(fill_q), 7)
                for u in tail_units[ic]:
                    fill_q.insert(pos, u)
                    pos += 1

    def emit_proj_unit(s, oc, li):
        st_ = S[s]
        pt = ps2.tile([P, 512], F32, tag="p2", name="proj_ps")
        for kcp in range(2):
            nc.tensor.matmul(pt,
                             c.pT[:, kcp, :, oc * P:(oc + 1) * P],
                             st_.att[kcp][:, :, li * 512:(li + 1) * 512],
                             start=(kcp == 0), stop=(kcp == 1),
                             perf_mode=PM.DoubleRow)
        ot = op_.tile([P, 512], F32, tag="ot", name="ot")
        nc.vector.tensor_scalar(ot, pt, 1.0 / WSCALE, c.pb[oc],
                                op0=OP.mult, op1=OP.add)
        nc.gpsimd.tensor_tensor(ot, ot, st_.x[oc][:, li * 512:(li + 1) * 512],
                                OP.add)
        for q in range(2):
            sl = slice(li * 512 + q * 256, li * 512 + (q + 1) * 256)
            next_q().dma_start(o_r[s, oc][:, sl], ot[:, q * 256:(q + 1) * 256])

    # ---------------- schedule ----------------
    # startup: bias rows (single-descriptor) then x s0 on the two fast HWDGE
    # queues; critical weight stages interleave; gpsimd's SWDGE queue only
    # gets work after its engine finishes the mask consts
    _consts_early(nc, c, const, nw_d, nb_d, qb_d, pb_d)
    emit_x_load(0, [nc.sync, nc.scalar])
    ws_first = {}
    for oc, eng in ((0, nc.sync), (4, nc.scalar), (8, nc.gpsimd),
                    (9, nc.gpsimd), (10, nc.sync), (11, nc.scalar)):
        ws_first[oc] = emit_tr_stage(oc, eng)
    emit_x_load(1, [nc.sync, nc.scalar])
    emit_bias_cols()
    emit_gn_stats(0)
    for oc in (0, 4):
        emit_tr_unit(oc, ws_first[oc])
    emit_gn_apply(0)
    emit_qkv_qk(0, 0)         # pair(0,0) q/k: its DVE epilogues gate the
    for oc in (8, 9, 10, 11):
        emit_tr_unit(oc, ws_first[oc])
    _consts_late(nc, c, const)
    emit_v(0, [0, 1, 2, 3])   # first S-matmuls, so they go before gn(1)
    emit_gn_stats(1)
    emit_gn_apply(1)

    # everything else becomes filler units popped per attention step; the
    # fill_q order encodes the just-in-time deadlines.  Units that may wait
    # until the sample-1 attention window go to the fill_q2 reserve, so the
    # PE stays fed (and at high p-state) through BOTH windows.
    for lc in range(4, LK):
        fill_q.append(lambda lc=lc: emit_v(0, [lc]))
    for oc_t, oc_a, oc_b in ((1, 1, 5), (2, 2, 6), (3, 3, 7)):
        fill_q.append(lambda oc=oc_t: emit_tr_unit(oc, emit_tr_stage(oc)))
        fill_q.append(lambda oc=oc_t: emit_tr_unit(oc + 4, emit_tr_stage(oc + 4)))
        for li in range(2):
            fill_q.append(lambda oc=oc_a, li=li: emit_qkv_unit(0, oc, li))
        for li in range(2):
            fill_q.append(lambda oc=oc_b, li=li: emit_qkv_unit(0, oc, li))
    for oc in (12, 13, 14, 15):       # proj weights, needed from pair(1,0)
        fill_q.append(lambda oc=oc: emit_tr_unit(oc, emit_tr_stage(oc)))
    for oc in (0, 4):                 # pair(1,0)'s q/k - before its S mms
        for li in range(2):
            fill_q.append(lambda oc=oc, li=li: emit_qkv_unit(1, oc, li))
    for lc in range(LK):              # all v(1) - consumed from pair(1,0) on
        fill_q.append(lambda lc=lc: emit_v(1, [lc]))
    for oc in (1, 5, 2, 6, 3, 7):     # later pairs' q/k ride the reserve
        for li in range(2):
            fill_q2.append(lambda oc=oc, li=li: emit_qkv_unit(1, oc, li))

    for hp in range(3):
        emit_pair(0, hp)
    emit_pair(0, 3, tail_units={
        0: [lambda oc=oc: emit_proj_unit(0, oc, 0) for oc in range(CK)],
        1: [lambda oc=oc: emit_proj_unit(0, oc, 1) for oc in range(CK)],
    })
    while fill_q:   # anything pair(1,0) needs that hasn't popped yet
        pop_fill()

    for hp in range(3):
        emit_pair(1, hp)
    emit_pair(1, 3, tail_units={
        0: [lambda oc=oc: emit_proj_unit(1, oc, 0) for oc in range(CK)],
        1: [lambda oc=oc: emit_proj_unit(1, oc, 1) for oc in range(CK)],
    })
    while fill_q or fill_q2:
        pop_fill()


def _build():
    if "nc" in _NC_CACHE:
        return _NC_CACHE["nc"]
    nc = bacc.Bacc("TRN2", target_bir_lowering=False, debug=False)
    x_d = nc.dram_tensor("x", (BPC, C, H, W), F32, kind="ExternalInput")
    nw_d = nc.dram_tensor("norm_w", (C,), F32, kind="ExternalInput")
    nb_d = nc.dram_tensor("norm_b", (C,), F32, kind="ExternalInput")
    qw_d = nc.dram_tensor("qkv_w", (3 * C, C), F32, kind="ExternalInput")
    qb_d = nc.dram_tensor("qkv_b", (3 * C,), F32, kind="ExternalInput")
    pw_d = nc.dram_tensor("proj_w", (C, C), F32, kind="ExternalInput")
    pb_d = nc.dram_tensor("proj_b", (C,), F32, kind="ExternalInput")
    out_d = nc.dram_tensor("out", (BPC, C, H, W), F32, kind="ExternalOutput")
    with tile.TileContext(nc) as tc:
        with (
            tc.tile_pool(name="const", bufs=1) as const,
            tc.tile_pool(name="stage", bufs=6) as stage,
            tc.tile_pool(name="xp", bufs=2) as xp,
            tc.tile_pool(name="hp", bufs=2) as hp_,
            tc.tile_pool(name="qkp", bufs=2) as qkp,
            tc.tile_pool(name="vp", bufs=2) as vp,
            tc.tile_pool(name="ep", bufs=3) as ep,
            tc.tile_pool(name="attp", bufs=2) as attp,
            tc.tile_pool(name="op", bufs=2) as op_,
            tc.tile_pool(name="sm", bufs=1) as sm,
            tc.tile_pool(name="csp", bufs=2) as csp,
            tc.tile_pool(name="ps", bufs=3, space="PSUM") as ps,
            tc.tile_pool(name="ps2", bufs=2, space="PSUM") as ps2,
        ):
            pools = (const, stage, xp, hp_, qkp, vp, ep, attp, op_, sm, csp, ps, ps2)
            _emit(nc, tc, pools, x_d, out_d, nw_d, nb_d, qw_d, qb_d, pw_d, pb_d)
    nc.compile()
    _NC_CACHE["nc"] = nc
    return nc


def kernel(x, norm_w, norm_b, qkv_w, qkv_b, proj_w, proj_b):
    x = np.ascontiguousarray(x, dtype=np.float32)
    args = {
        "norm_w": np.ascontiguousarray(norm_w, np.float32),
        "norm_b": np.ascontiguousarray(norm_b, np.float32),
        "qkv_w": np.ascontiguousarray(qkv_w, np.float32),
        "qkv_b": np.ascontiguousarray(qkv_b, np.float32),
        "proj_w": np.ascontiguousarray(proj_w, np.float32),
        "proj_b": np.ascontiguousarray(proj_b, np.float32),
    }
    nc = _build()
    in_maps = [dict(args, x=x[i * BPC:(i + 1) * BPC]) for i in range(N_CORES)]
    res = run_bass_kernel_spmd(nc, in_maps, core_ids=list(range(N_CORES)))
    return np.concatenate([r["out"] for r in res.results], axis=0)


# revision 49
# speedup vs baseline: 1.2175x; 1.0223x over previous
"""AttentionBlock (GroupNorm + 8-head self-attention + proj + residual) on 8 trn2 cores.

Sharding: data-parallel over batch B=16 -> 2 samples per core. No collectives.

Per-sample dataflow (C=512 channels, L=1024 pixels, 8 heads x 64 dims):
  - x (C, L) lives as 4 SBUF f32 tiles (128, 1024), channels on partitions; x
    stays resident until the proj residual add (no re-load).
  - GroupNorm: per-channel mean/var via bn_stats over L; 16-channel group
    aggregation + broadcast-back via tiny mask matmuls on the PE; rstd via a
    DVE-only rsqrt bit-hack (keeps the ACT engine exp-table resident, no
    table swaps).  h is written as fp8 channel-chunk-pair tiles (128,2,1024).
  - All four big GEMMs (qkv, v, attention AV, proj) run in fp8e4 DoubleRow
    perf mode - each matmul contracts TWO 128-row K-tiles at 0.5 cycles/row.
    Weights are pre-scaled x8 into fp8 (avoids subnormals), epilogues fold
    the /8 back in.  Only the S=K^T Q matmuls stay bf16 (their K=64
    contraction can't pair, and fp8 would add noise for no speed).
  - Attention per head pair, split by i-halves so PSUM double-buffers:
    S^T in bf16 (row-packed head pairs share the PE, K=64 each); exp on
    ScalarE with the 1/8 scale and a fixed -3 bias fused (cancels in the
    softmax ratio, keeps fp8 e < 240), writing fp8 e-pair tiles; AV
    DoubleRow-contracts both jc chunks of a pair, with the softmax
    denominator riding along as PSUM row 64.  Attention outputs are stored
    as raw/64 in fp8 (range safety); the denominator reciprocal (x64,
    reciprocal_approx_fast) is broadcast back per i-half via a K=2 selector
    matmul and one normalization multiply, emitted per half so the last
    pair's proj can start while its second half still runs.
  - proj + bias + residual, write out split across two DMA queues.
  - Cross-sample software pipeline: sample s+1's groupnorm/QKV/V fill the PE
    while ScalarE works through sample s's exps; sample s's proj fills the
    head of sample s+1's attention.

Startup: x and the six critical weight stages load on the two fast HWDGE
queues (sync/scalar) split in halves, with the gpsimd SWDGE queue taking the
second halves; first attention matmul starts ~13us in.
"""

import numpy as np

import concourse.bass as bass
import concourse.mybir as mybir
import concourse.tile as tile
from concourse import bacc
from concourse.bass_utils import run_bass_kernel_spmd
from concourse.masks import make_identity

F32 = mybir.dt.float32
F32R = mybir.dt.float32r
BF16 = mybir.dt.bfloat16
F8 = mybir.dt.float8e4
U32 = mybir.dt.uint32
AF = mybir.ActivationFunctionType
OP = mybir.AluOpType
PM = mybir.MatmulPerfMode

B, C, H, W = 16, 512, 32, 32
L = H * W
NH, HD = 8, 64
NG, GS = 32, 16
EPS = 1e-5
N_CORES = 8
BPC = B // N_CORES  # samples per core
P = 128
CK = C // P   # 4 channel chunks
LK = L // P   # 8 pixel chunks
SCALE = HD ** -0.5
ESHIFT = -3.0   # exp(x*scale + ESHIFT): cancels in softmax, keeps e < fp8 max
WSCALE = 8.0    # weights pre-scaled into fp8; epilogues multiply by 1/WSCALE
ADIV = 64.0     # attention outputs stored as raw/ADIV in fp8; rsum carries xADIV
RSQRT_MAGIC = 0x5F3759DF

_NC_CACHE = {}


class Ctx:
    pass


def _consts_early(nc, c, const, nw_d, nb_d, qb_d, pb_d):
    # bias vectors load as single-descriptor ROWS (a (128,1)-column DMA costs
    # ~1.4us of queue time; a contiguous row is free) - PE transposes turn
    # them into per-partition columns right after the identity exists
    c.nwrow = const.tile([1, C], F32, tag="nwrow")
    c.nbrow = const.tile([1, C], F32, tag="nbrow")
    c.pbrow = const.tile([1, C], F32, tag="pbrow")
    c.qbrow = const.tile([1, 3 * C], F32, tag="qbrow")
    nc.sync.dma_start(c.nwrow, nw_d.ap()[None, :])
    nc.sync.dma_start(c.nbrow, nb_d.ap()[None, :])
    nc.sync.dma_start(c.pbrow, pb_d.ap()[None, :])
    nc.sync.dma_start(c.qbrow, qb_d.ap()[None, :])

    # only what groupnorm + the first transposes need; everything else is
    # deferred so it can't block the DVE/gpsimd in-order queues at startup
    c.ident = const.tile([P, P], F32, tag="ident")
    make_identity(nc, c.ident)

    # gmask[kc][ch, g] = 1/16 iff global_channel // 16 == g   (128, 32)
    c.gmask = []
    for kc in range(CK):
        gm = const.tile([P, NG], F32, tag=f"gmask{kc}", name=f"gmask{kc}")
        nc.gpsimd.memset(gm, 1.0 / GS)
        nc.gpsimd.affine_select(
            out=gm, in_=gm, compare_op=OP.is_ge, fill=0.0,
            base=P * kc, channel_multiplier=1, pattern=[[-GS, NG]])
        nc.gpsimd.affine_select(
            out=gm, in_=gm, compare_op=OP.is_ge, fill=0.0,
            base=(GS - 1) - P * kc, channel_multiplier=-1, pattern=[[GS, NG]])
        c.gmask.append(gm)

    # bmask[g, ch] = 1 iff ch // 16 == g  (32, 512)
    c.bmask = const.tile([NG, C], F32, tag="bmask")
    nc.gpsimd.memset(c.bmask, 1.0)
    nc.gpsimd.affine_select(
        out=c.bmask, in_=c.bmask, compare_op=OP.is_ge, fill=0.0,
        base=0, channel_multiplier=-GS, pattern=[[1, C]])
    nc.gpsimd.affine_select(
        out=c.bmask, in_=c.bmask, compare_op=OP.is_ge, fill=0.0,
        base=GS - 1, channel_multiplier=GS, pattern=[[-1, C]])

    c.magic = const.tile([NG, 1], U32, tag="magic")
    nc.vector.memset(c.magic, RSQRT_MAGIC)
    c.eshift = const.tile([P, 1], F32, tag="eshift")
    nc.vector.memset(c.eshift, ESHIFT)
    c.ones1 = const.tile([1, P], F32, tag="ones1")
    nc.vector.memset(c.ones1, 1.0)


def _consts_late(nc, c, const):
    # sel2[h2, ch] = 1 iff ch // 64 == h2  (2, 128), f32r for full-rate matmul
    sel2s = const.tile([2, P], F32, tag="sel2s")
    nc.gpsimd.memset(sel2s, 1.0)
    nc.gpsimd.affine_select(
        out=sel2s, in_=sel2s, compare_op=OP.is_ge, fill=0.0,
        base=0, channel_multiplier=-HD, pattern=[[1, P]])
    nc.gpsimd.affine_select(
        out=sel2s, in_=sel2s, compare_op=OP.is_ge, fill=0.0,
        base=HD - 1, channel_multiplier=HD, pattern=[[-1, P]])
    c.sel2 = const.tile([2, P], F32R, tag="sel2")
    nc.vector.tensor_copy(out=c.sel2, in_=sel2s)


def _emit(nc, tc, pools, x_d, out_d, nw_d, nb_d, qw_d, qb_d, pw_d, pb_d):
    (const, stage, xp, hp_, qkp, vp, ep, attp, op_, sm, csp, ps, ps2) = pools

    qi = Ctx()
    qi.steady_i = 0

    def next_q():
        e = [nc.sync, nc.gpsimd][qi.steady_i % 2]
        qi.steady_i += 1
        return e

    c = Ctx()

    x_r = x_d.ap().rearrange("b (kc p) h w -> b kc p (h w)", p=P)
    o_r = out_d.ap().rearrange("b (kc p) h w -> b kc p (h w)", p=P)

    S = [Ctx() for _ in range(BPC)]
    for st_ in S:
        st_.x = [None] * CK

    def emit_x_load(s, engines, split=True):
        # split each (128, 1024) tile into two partition halves spread over
        # the queues so every kc chunk completes early
        st_ = S[s]
        for kc in range(CK):
            xt = xp.tile([P, L], F32, tag=f"x{kc}", name=f"x{kc}_{s}")
            if split:
                for half in range(2):
                    sl = slice(half * 64, (half + 1) * 64)
                    engines[(2 * kc + half) % len(engines)].dma_start(
                        xt[sl, :], x_r[s, kc][sl, :])
            else:
                engines[kc % len(engines)].dma_start(xt, x_r[s, kc])
            st_.x[kc] = xt

    def emit_bias_cols():
        # transpose the bias rows into per-partition columns in one PSUM
        # pass; broadcast the v bias via a K=1 ones-matmul
        bp = ps2.tile([P, 512], F32, tag="p2", name="bias_ps")
        one = c.ones1[0:1, 0:1]   # 1x1 identity for single-row transposes
        for kc in range(CK):
            nc.tensor.transpose(bp[:, kc:kc + 1],
                                c.nwrow[:, kc * P:(kc + 1) * P], one)
            nc.tensor.transpose(bp[:, 4 + kc:5 + kc],
                                c.nbrow[:, kc * P:(kc + 1) * P], one)
            nc.tensor.transpose(bp[:, 16 + kc:17 + kc],
                                c.pbrow[:, kc * P:(kc + 1) * P], one)
        for oc in range(8):
            nc.tensor.transpose(bp[:, 8 + oc:9 + oc],
                                c.qbrow[:, oc * P:(oc + 1) * P], one)
        bias_cols = const.tile([P, 20], F32, tag="bias_cols")
        nc.vector.tensor_copy(out=bias_cols, in_=bp[:, 0:20])
        c.nw = [bias_cols[:, kc:kc + 1] for kc in range(CK)]
        c.nb = [bias_cols[:, 4 + kc:5 + kc] for kc in range(CK)]
        c.qb = [bias_cols[:, 8 + oc:9 + oc] for oc in range(8)]
        c.pb = [bias_cols[:, 16 + kc:17 + kc] for kc in range(CK)]

    def emit_vb():
        vps = ps2.tile([P, 512], F32, tag="p2", name="vb_ps")
        nc.tensor.matmul(vps, c.ones1, c.qbrow[:, 1024:1536],
                         start=True, stop=True)
        c.vb = const.tile([P, 512], F32, tag="vb")
        nc.vector.tensor_copy(out=c.vb, in_=vps)

    def emit_gn_stats(s, kcs=None):
        st_ = S[s]
        if not hasattr(st_, "stat2"):
            st_.stat2 = [None] * CK
        for kc in (range(CK) if kcs is None else kcs):
            xt = st_.x[kc]
            bst = sm.tile([P, 2, 6], F32, tag="bst", name="bst")
            nc.vector.bn_stats(out=bst[:, 0, :], in_=xt[:, 0:512])
            nc.vector.bn_stats(out=bst[:, 1, :], in_=xt[:, 512:1024])
            mv = sm.tile([P, 2], F32, tag="mv", name="mv")
            nc.vector.bn_aggr(out=mv, in_=bst)
            st2 = sm.tile([P, 2], F32, tag="st2", name="st2")
            nc.vector.tensor_copy(out=st2[:, 0:1], in_=mv[:, 0:1])
            nc.vector.tensor_tensor(st2[:, 1:2], mv[:, 0:1], mv[:, 0:1], OP.mult)
            nc.vector.tensor_tensor(st2[:, 1:2], st2[:, 1:2], mv[:, 1:2], OP.add)
            st_.stat2[kc] = st2

    def emit_rsqrt(dst, var):
        # dst = (var + EPS) ** -0.5 entirely on DVE: shift-subtract seed +
        # two Newton-Raphson passes (~1e-5 rel) - keeps ACT's exp table hot
        vv = sm.tile([NG, 1], F32, tag="vv", name="vv")
        y = sm.tile([NG, 1], F32, tag="y", name="y")
        t1 = sm.tile([NG, 1], F32, tag="t1", name="t1")
        nc.vector.tensor_scalar(vv, var, EPS, None, op0=OP.add)
        nc.vector.tensor_scalar(y.bitcast(U32), vv.bitcast(U32), 1, None,
                                op0=OP.logical_shift_right)
        nc.vector.tensor_tensor(y.bitcast(U32), c.magic, y.bitcast(U32),
                                OP.subtract)
        for _ in range(2):
            nc.vector.tensor_tensor(t1, vv, y, OP.mult)
            nc.vector.tensor_tensor(t1, t1, y, OP.mult)
            nc.vector.tensor_scalar(t1, t1, -0.5, 1.5, op0=OP.mult, op1=OP.add)
            nc.vector.tensor_tensor(y, y, t1, OP.mult)
        nc.vector.tensor_copy(out=dst, in_=y)

    def emit_gn_apply(s):
        st_ = S[s]
        gps = ps2.tile([P, 512], F32, tag="p2", name="gn_ps")
        for kc in range(CK):
            nc.tensor.matmul(gps[0:NG, 0:2], c.gmask[kc], st_.stat2[kc],
                             start=(kc == 0), stop=(kc == CK - 1))
        gst = sm.tile([NG, 2], F32, tag="gst", name="gst")
        gsb = sm.tile([NG, 2], F32, tag="gsb", name="gsb")
        gtmp = sm.tile([NG, 1], F32, tag="gtmp", name="gtmp")
        nc.vector.tensor_copy(out=gsb, in_=gps[0:NG, 0:2])
        nc.vector.tensor_tensor(gtmp, gsb[:, 0:1], gsb[:, 0:1], OP.mult)
        nc.vector.tensor_tensor(gtmp, gsb[:, 1:2], gtmp, OP.subtract)  # var
        emit_rsqrt(gst[:, 1:2], gtmp)                                  # rstd
        nc.vector.tensor_copy(out=gst[:, 0:1], in_=gsb[:, 0:1])       # gmean
        chps = ps2.tile([P, 512], F32, tag="p2", name="gn_ps2")
        for kc in range(CK):
            nc.tensor.matmul(chps[:, kc * 2: kc * 2 + 2],
                             c.bmask[:, kc * P:(kc + 1) * P], gst,
                             start=True, stop=True)
        st_.h = [None, None]   # fp8 channel-chunk-pair tiles (128, 2, 1024)
        for kc in range(CK):
            Acol = sm.tile([P, 1], F32, tag="Acol", name="Acol")
            Bcol = sm.tile([P, 1], F32, tag="Bcol", name="Bcol")
            nc.vector.tensor_tensor(Acol, chps[:, kc * 2 + 1: kc * 2 + 2],
                                    c.nw[kc], OP.mult)
            nc.vector.tensor_tensor(Bcol, chps[:, kc * 2: kc * 2 + 1], Acol, OP.mult)
            nc.vector.tensor_tensor(Bcol, c.nb[kc], Bcol, OP.subtract)
            kcp, t = kc // 2, kc % 2
            if st_.h[kcp] is None:
                st_.h[kcp] = hp_.tile([P, 2, L], F8, tag=f"h{kcp}",
                                      name=f"h{kcp}_{s}")
            # alternate DVE/GpSimd so the four h writes take ~2 writes of
            # wall time on the first-exp critical path
            eng = nc.vector if kc % 2 else nc.gpsimd
            with nc.allow_low_precision(reason="fp8 activations"):
                eng.tensor_scalar(st_.h[kcp][:, t, :], st_.x[kc],
                                  Acol, Bcol, op0=OP.mult, op1=OP.add)
        st_.qkT = [None] * 8
        st_.v = [None] * (LK // 2)
        st_.att = [None, None]  # fp8 pair tiles (128, 2, 1024), t = hp % 2

    qw_r4 = qw_d.ap().rearrange("(oc p) ch -> oc p ch", p=P)
    pw_r4 = pw_d.ap().rearrange("(oc p) ch -> oc p ch", p=P)
    # wT: (128, kcp, t, col) fp8, pre-scaled by WSCALE.  col < 1536 for qkv.
    c.wT = const.tile([P, 2, 2, 3 * C], F8, tag="wT", name="wT")
    c.pT = const.tile([P, 2, 2, C], F8, tag="pT", name="pT")

    def emit_tr_stage(oc, eng=None):
        src_r = qw_r4[oc] if oc < 12 else pw_r4[oc - 12]
        ws = stage.tile([P, C], F32, tag="wstage", name="wstage")
        (eng or next_q()).dma_start(ws, src_r)
        return ws

    def emit_tr_unit(oc, ws, on_act=False):
        col = (oc if oc < 12 else oc - 12) * P
        pt = ps2.tile([P, 512], F32, tag="p2", name="tr_ps")
        for kc in range(CK):
            nc.tensor.transpose(pt[:, kc * P:(kc + 1) * P],
                                ws[:, kc * P:(kc + 1) * P], c.ident)
        dst = c.wT if oc < 12 else c.pT
        dst_ap = dst[:, :, :, col:col + P]
        src_ap = pt.rearrange("p (a b i) -> p a b i", a=2, i=P)
        with nc.allow_low_precision(reason="fp8 weights, x8 prescaled"):
            if on_act:
                # startup only: ACT is idle pre-attention, and this keeps the
                # x8-cast copies out of the groupnorm-critical DVE queue
                nc.scalar.activation(dst_ap, src_ap, AF.Copy, scale=WSCALE)
            else:
                nc.vector.tensor_scalar(dst_ap, src_ap, WSCALE, None,
                                        op0=OP.mult)

    def emit_qkv_unit(s, oc, li):
        st_ = S[s]
        if st_.qkT[oc] is None:
            st_.qkT[oc] = qkp.tile([P, L], BF16, tag=f"qk{oc}", name=f"qk{oc}_{s}")
        dst = st_.qkT[oc]
        pt = ps2.tile([P, 512], F32, tag="p2", name="qkv_ps")
        for kcp in range(2):
            nc.tensor.matmul(pt,
                             c.wT[:, kcp, :, oc * P:(oc + 1) * P],
                             st_.h[kcp][:, :, li * 512:(li + 1) * 512],
                             start=(kcp == 0), stop=(kcp == 1),
                             perf_mode=PM.DoubleRow)
        nc.vector.tensor_scalar(dst[:, li * 512:(li + 1) * 512],
                                pt, 1.0 / WSCALE, c.qb[oc],
                                op0=OP.mult, op1=OP.add)

    def emit_qkv_qk(s, hp):
        for oc in (hp, 4 + hp):
            for li in range(2):
                emit_qkv_unit(s, oc, li)

    def emit_v(s, lcs):
        # v pair tiles: (128 jpix, 2 chunk-parity, 8 heads, 64+1+3pad) fp8;
        # head pitch 68 keeps DoubleRow ldweights panels 4-byte aligned
        st_ = S[s]
        for lc in lcs:
            lcp, t = lc // 2, lc % 2
            pt = ps2.tile([P, 512], F32, tag="p2", name="v_ps")
            for kcp in range(2):
                nc.tensor.matmul(pt,
                                 st_.h[kcp][:, :, lc * P:(lc + 1) * P],
                                 c.wT[:, kcp, :, 1024:1536],
                                 start=(kcp == 0), stop=(kcp == 1),
                                 perf_mode=PM.DoubleRow)
            if st_.v[lcp] is None:
                vt = vp.tile([P, 2, NH, HD + 4], F8, tag=f"v{lcp}",
                             name=f"v{lcp}_{s}")
                nc.vector.memset(vt[:, :, :, HD:HD + 1], 1.0)
                nc.vector.memset(vt[:, :, :, HD + 1:HD + 4], 0.0)
                st_.v[lcp] = vt
            vt = st_.v[lcp]
            with nc.allow_low_precision(reason="fp8 attention values"):
                nc.vector.scalar_tensor_tensor(
                    out=vt[:, t, :, 0:HD],
                    in0=pt.rearrange("p (h d) -> p h d", d=HD),
                    scalar=1.0 / WSCALE,
                    in1=c.vb.rearrange("p (h d) -> p h d", d=HD),
                    op0=OP.mult, op1=OP.add)

    fill_q = []    # units for the sample-0 attention window (drains first)
    fill_q2 = []   # reserve units held back for the sample-1 window

    def pop_fill(n=1):
        for _ in range(n):
            if fill_q:
                fill_q.pop(0)()
            elif fill_q2:
                fill_q2.pop(0)()

    def make_norm2(s, hp, li, rsum):
        st_ = S[s]

        def norm2():
            rb2 = ps2.tile([P, 512], F32, tag="p2", name="rb2_ps")
            nc.tensor.matmul(rb2, c.sel2, rsum[:, li * 512:(li + 1) * 512],
                             start=True, stop=True)
            kcp, t = hp // 2, hp % 2
            sl = st_.att[kcp][:, t, li * 512:(li + 1) * 512]
            with nc.allow_low_precision(reason="fp8 attention probs"):
                nc.vector.tensor_tensor(sl, sl, rb2, OP.mult)
        return norm2

    carry = Ctx()
    carry.stile = None

    def s_mms_for(s2, hp2, ic, jc):
        st2 = S[s2]
        kT, qT = st2.qkT[4 + hp2], st2.qkT[hp2]
        stile = ps.tile([P, 1024], F32, tag="s", name=f"s_{hp2}_{ic}_{jc}")
        for h2 in range(2):
            nc.tensor.matmul(
                stile[:, h2 * 512:(h2 + 1) * 512],
                kT[h2 * HD:(h2 + 1) * HD, jc * P:(jc + 1) * P],
                qT[h2 * HD:(h2 + 1) * HD, ic * 512:(ic + 1) * 512],
                start=True, stop=True)
        return stile

    def emit_pair(s, hp, tail_units=None):
        st_ = S[s]
        kcp_a, t_a = hp // 2, hp % 2
        if st_.att[kcp_a] is None:
            st_.att[kcp_a] = attp.tile([P, 2, L], F8, tag=f"att{kcp_a}",
                                       name=f"att{kcp_a}_{s}")
        csum = csp.tile([2, L], F32, tag="csum", name=f"csum_{s}_{hp}")

        def s_mms(ic, jc):
            return s_mms_for(s, hp, ic, jc)

        def next_block(ic):
            # the block whose first S-matmuls we prefetch during this
            # block's last step, so the exp stream never waits at a boundary
            if ic == 0:
                return (s, hp, 1)
            if hp < 3 and S[s].qkT[hp + 5] is not None:
                return (s, hp + 1, 0)
            return None

        for ic in range(2):
            av = ps.tile([P, 1024], F32, tag="s", name=f"av_{hp}_{ic}")
            if carry.stile is not None:
                stile, carry.stile = carry.stile, None
            else:
                stile = s_mms(ic, 0)
            for jcp in range(LK // 2):
                e_t = ep.tile([P, 2, 1024], F8, tag="e", name="e_t")
                for t in range(2):
                    jc = 2 * jcp + t
                    nc.scalar.activation(e_t[:, t, :], stile, AF.Exp,
                                         scale=SCALE, bias=c.eshift)
                    # emit next S ahead of this AV so the PE stream runs one
                    # step ahead of ScalarE; soak the PE with filler units
                    if jc + 1 < LK:
                        stile = s_mms(ic, jc + 1)
                    else:
                        nb = next_block(ic)
                        if nb is not None:
                            carry.stile = s_mms_for(nb[0], nb[1], nb[2], 0)
                    pop_fill()
                for h2 in range(2):
                    nc.tensor.matmul(
                        av[0:HD + 2, h2 * 512:(h2 + 1) * 512],
                        st_.v[jcp][:, :, 2 * hp + h2, 0:HD + 2],
                        e_t[:, :, h2 * 512:(h2 + 1) * 512],
                        start=(jcp == 0), stop=(jcp == LK // 2 - 1),
                        perf_mode=PM.DoubleRow)
            for h2 in range(2):
                with nc.allow_low_precision(reason="fp8, /64 range guard"):
                    nc.vector.tensor_scalar(
                        st_.att[kcp_a][h2 * HD:(h2 + 1) * HD, t_a,
                                       ic * 512:(ic + 1) * 512],
                        av[0:HD, h2 * 512:(h2 + 1) * 512],
                        1.0 / ADIV, None, op0=OP.mult)
                cstage = sm.tile([1, 512], F32, tag="cstage", name="cstage")
                nc.vector.tensor_copy(
                    out=cstage, in_=av[HD:HD + 1, h2 * 512:(h2 + 1) * 512])
                # NOTE: never issue DMAs from the scalar engine - any HWDGE
                # work on ACT slows every activation ~20% for the whole run
                nc.sync.dma_start(csum[h2:h2 + 1, ic * 512:(ic + 1) * 512], cstage)
            # per-half denominators: approx reciprocal on DVE, x64 rounding
            # copy (f32r for the selector matmul) on gpsimd (DVE for the
            # tail-critical second sample), then the normalize unit queues
            rscr = csp.tile([2, 512], F32, tag="rscr", name=f"rscr_{s}_{hp}_{ic}")
            rsum = csp.tile([2, L], F32R, tag="rsum", name=f"rsum_{s}_{hp}",
                            bufs=2) if ic == 0 else st_.rsum_cur
            st_.rsum_cur = rsum
            with nc.allow_low_precision(reason="softmax denominators"):
                nc.vector.reciprocal_approx_fast(
                    out=rscr, in_=csum[:, ic * 512:(ic + 1) * 512])
            nc.vector.tensor_scalar(rsum[:, ic * 512:(ic + 1) * 512], rscr,
                                    ADIV, None, op0=OP.mult)
            fill_q.insert(min(len(fill_q), 6), make_norm2(s, hp, ic, rsum))
            if tail_units and tail_units.get(ic):
                pos = min(len(fill_q), 7)
                for u in tail_units[ic]:
                    fill_q.insert(pos, u)
                    pos += 1

    def emit_proj_unit(s, oc, li):
        st_ = S[s]
        pt = ps2.tile([P, 512], F32, tag="p2", name="proj_ps")
        for kcp in range(2):
            nc.tensor.matmul(pt,
                             c.pT[:, kcp, :, oc * P:(oc + 1) * P],
                             st_.att[kcp][:, :, li * 512:(li + 1) * 512],
                             start=(kcp == 0), stop=(kcp == 1),
                             perf_mode=PM.DoubleRow)
        ot = op_.tile([P, 512], F32, tag="ot", name="ot")
        nc.vector.tensor_scalar(ot, pt, 1.0 / WSCALE, c.pb[oc],
                                op0=OP.mult, op1=OP.add)
        nc.gpsimd.tensor_tensor(ot, ot, st_.x[oc][:, li * 512:(li + 1) * 512],
                                OP.add)
        for q in range(2):
            sl = slice(li * 512 + q * 256, li * 512 + (q + 1) * 256)
            next_q().dma_start(o_r[s, oc][:, sl], ot[:, q * 256:(q + 1) * 256])

    # ---------------- schedule ----------------
    # startup: bias rows (single-descriptor) then x s0 exclusively on the two
    # fast HWDGE queues; ALL weight stages ride the gpsimd SWDGE queue (its
    # engine builds the mask consts first, then fires the triggers); x s1
    # trails x s0.  Sample-1 groupnorm runs as filler units so it can't
    # steal DVE time from the sample-0 critical chain.
    _consts_early(nc, c, const, nw_d, nb_d, qb_d, pb_d)
    emit_x_load(0, [nc.sync, nc.scalar])
    ws_first = {}
    for oc in (0, 4, 8, 9, 10, 11):
        ws_first[oc] = emit_tr_stage(oc, nc.gpsimd)
    emit_x_load(1, [nc.sync, nc.scalar])
    emit_gn_stats(0)
    emit_bias_cols()
    for oc in (0, 4):
        emit_tr_unit(oc, ws_first[oc])
    emit_gn_apply(0)
    emit_qkv_qk(0, 0)         # pair(0,0) q/k: its DVE epilogues gate the
    for oc in (8, 9, 10, 11):
        emit_tr_unit(oc, ws_first[oc])
    emit_vb()
    _consts_late(nc, c, const)
    emit_v(0, [0, 1, 2, 3])   # first S-matmuls gate on these

    # everything else becomes filler units popped per attention step; the
    # fill_q order encodes the just-in-time deadlines.  Units that may wait
    # until the sample-1 attention window go to the fill_q2 reserve, so the
    # PE stays fed (and at high p-state) through BOTH windows.
    for lc in (4, 5):
        fill_q.append(lambda lc=lc: emit_v(0, [lc]))
    for kc in range(CK):
        fill_q.append(lambda kc=kc: emit_gn_stats(1, [kc]))
    for lc in (6, 7):
        fill_q.append(lambda lc=lc: emit_v(0, [lc]))
    fill_q.append(lambda: emit_gn_apply(1))
    for oc_t, oc_a, oc_b in ((1, 1, 5), (2, 2, 6), (3, 3, 7)):
        fill_q.append(lambda oc=oc_t: emit_tr_unit(oc, emit_tr_stage(oc)))
        fill_q.append(lambda oc=oc_t: emit_tr_unit(oc + 4, emit_tr_stage(oc + 4)))
        for li in range(2):
            fill_q.append(lambda oc=oc_a, li=li: emit_qkv_unit(0, oc, li))
        for li in range(2):
            fill_q.append(lambda oc=oc_b, li=li: emit_qkv_unit(0, oc, li))
    for oc in (12, 13, 14, 15):       # proj weights, needed from pair(1,0)
        fill_q.append(lambda oc=oc: emit_tr_unit(oc, emit_tr_stage(oc)))
    for oc in (0, 4):                 # pair(1,0)'s q/k - before its S mms
        for li in range(2):
            fill_q.append(lambda oc=oc, li=li: emit_qkv_unit(1, oc, li))
    for lc in range(LK):              # all v(1) - consumed from pair(1,0) on
        fill_q.append(lambda lc=lc: emit_v(1, [lc]))
    for oc in (1, 5, 2, 6, 3, 7):     # later pairs' q/k ride the reserve
        for li in range(2):
            fill_q2.append(lambda oc=oc, li=li: emit_qkv_unit(1, oc, li))

    for hp in range(3):
        emit_pair(0, hp)
    emit_pair(0, 3, tail_units={
        0: [lambda oc=oc: emit_proj_unit(0, oc, 0) for oc in range(CK)],
        1: [lambda oc=oc: emit_proj_unit(0, oc, 1) for oc in range(CK)],
    })
    while fill_q:   # anything pair(1,0) needs that hasn't popped yet
        pop_fill()

    for hp in range(3):
        emit_pair(1, hp)
    emit_pair(1, 3, tail_units={
        0: [lambda oc=oc: emit_proj_unit(1, oc, 0) for oc in range(CK)],
        1: [lambda oc=oc: emit_proj_unit(1, oc, 1) for oc in range(CK)],
    })
    while fill_q or fill_q2:
        pop_fill()


def _build():
    if "nc" in _NC_CACHE:
        return _NC_CACHE["nc"]
    nc = bacc.Bacc("TRN2", target_bir_lowering=False, debug=False)
    x_d = nc.dram_tensor("x", (BPC, C, H, W), F32, kind="ExternalInput")
    nw_d = nc.dram_tensor("norm_w", (C,), F32, kind="ExternalInput")
    nb_d = nc.dram_tensor("norm_b", (C,), F32, kind="ExternalInput")
    qw_d = nc.dram_tensor("qkv_w", (3 * C, C), F32, kind="ExternalInput")
    qb_d = nc.dram_tensor("qkv_b", (3 * C,), F32, kind="ExternalInput")
    pw_d = nc.dram_tensor("proj_w", (C, C), F32, kind="ExternalInput")
    pb_d = nc.dram_tensor("proj_b", (C,), F32, kind="ExternalInput")
    out_d = nc.dram_tensor("out", (BPC, C, H, W), F32, kind="ExternalOutput")
    with tile.TileContext(nc) as tc:
        with (
            tc.tile_pool(name="const", bufs=1) as const,
            tc.tile_pool(name="stage", bufs=6) as stage,
            tc.tile_pool(name="xp", bufs=2) as xp,
            tc.tile_pool(name="hp", bufs=2) as hp_,
            tc.tile_pool(name="qkp", bufs=2) as qkp,
            tc.tile_pool(name="vp", bufs=2) as vp,
            tc.tile_pool(name="ep", bufs=3) as ep,
            tc.tile_pool(name="attp", bufs=2) as attp,
            tc.tile_pool(name="op", bufs=2) as op_,
            tc.tile_pool(name="sm", bufs=1) as sm,
            tc.tile_pool(name="csp", bufs=2) as csp,
            tc.tile_pool(name="ps", bufs=3, space="PSUM") as ps,
            tc.tile_pool(name="ps2", bufs=2, space="PSUM") as ps2,
        ):
            pools = (const, stage, xp, hp_, qkp, vp, ep, attp, op_, sm, csp, ps, ps2)
            _emit(nc, tc, pools, x_d, out_d, nw_d, nb_d, qw_d, qb_d, pw_d, pb_d)
    nc.compile()
    _NC_CACHE["nc"] = nc
    return nc


def kernel(x, norm_w, norm_b, qkv_w, qkv_b, proj_w, proj_b):
    x = np.ascontiguousarray(x, dtype=np.float32)
    args = {
        "norm_w": np.ascontiguousarray(norm_w, np.float32),
        "norm_b": np.ascontiguousarray(norm_b, np.float32),
        "qkv_w": np.ascontiguousarray(qkv_w, np.float32),
        "qkv_b": np.ascontiguousarray(qkv_b, np.float32),
        "proj_w": np.ascontiguousarray(proj_w, np.float32),
        "proj_b": np.ascontiguousarray(proj_b, np.float32),
    }
    nc = _build()
    in_maps = [dict(args, x=x[i * BPC:(i + 1) * BPC]) for i in range(N_CORES)]
    res = run_bass_kernel_spmd(nc, in_maps, core_ids=list(range(N_CORES)))
    return np.concatenate([r["out"] for r in res.results], axis=0)


# revision 52
# speedup vs baseline: 1.2432x; 1.0211x over previous
"""AttentionBlock (GroupNorm + 8-head self-attention + proj + residual) on 8 trn2 cores.

Sharding: data-parallel over batch B=16 -> 2 samples per core. No collectives.

Per-sample dataflow (C=512 channels, L=1024 pixels, 8 heads x 64 dims):
  - x (C, L) lives as 4 SBUF f32 tiles (128, 1024), channels on partitions; x
    stays resident until the proj residual add (no re-load).
  - GroupNorm: per-channel mean/var via bn_stats over L; 16-channel group
    aggregation + broadcast-back via tiny mask matmuls on the PE; rstd via a
    DVE-only rsqrt bit-hack (keeps the ACT engine exp-table resident, no
    table swaps).  h is written as fp8 channel-chunk-pair tiles (128,2,1024).
  - All four big GEMMs (qkv, v, attention AV, proj) run in fp8e4 DoubleRow
    perf mode - each matmul contracts TWO 128-row K-tiles at 0.5 cycles/row.
    Weights are pre-scaled x8 into fp8 (avoids subnormals), epilogues fold
    the /8 back in.  Only the S=K^T Q matmuls stay bf16 (their K=64
    contraction can't pair, and fp8 would add noise for no speed).
  - Attention per head pair, split by i-halves so PSUM double-buffers:
    S^T in bf16 (row-packed head pairs share the PE, K=64 each); exp on
    ScalarE with the 1/8 scale and a fixed -3 bias fused (cancels in the
    softmax ratio, keeps fp8 e < 240), writing fp8 e-pair tiles; AV
    DoubleRow-contracts both jc chunks of a pair, with the softmax
    denominator riding along as PSUM row 64.  Attention outputs are stored
    as raw/64 in fp8 (range safety); the denominator reciprocal (x64,
    reciprocal_approx_fast) is broadcast back per i-half via a K=2 selector
    matmul and one normalization multiply, emitted per half so the last
    pair's proj can start while its second half still runs.
  - proj + bias + residual, write out split across two DMA queues.
  - Cross-sample software pipeline: sample s+1's groupnorm/QKV/V fill the PE
    while ScalarE works through sample s's exps; sample s's proj fills the
    head of sample s+1's attention.

Startup: x and the six critical weight stages load on the two fast HWDGE
queues (sync/scalar) split in halves, with the gpsimd SWDGE queue taking the
second halves; first attention matmul starts ~13us in.
"""

import numpy as np

import concourse.bass as bass
import concourse.mybir as mybir
import concourse.tile as tile
from concourse import bacc
from concourse.bass_utils import run_bass_kernel_spmd
from concourse.masks import make_identity

F32 = mybir.dt.float32
F32R = mybir.dt.float32r
BF16 = mybir.dt.bfloat16
F8 = mybir.dt.float8e4
U32 = mybir.dt.uint32
AF = mybir.ActivationFunctionType
OP = mybir.AluOpType
PM = mybir.MatmulPerfMode

B, C, H, W = 16, 512, 32, 32
L = H * W
NH, HD = 8, 64
NG, GS = 32, 16
EPS = 1e-5
N_CORES = 8
BPC = B // N_CORES  # samples per core
P = 128
CK = C // P   # 4 channel chunks
LK = L // P   # 8 pixel chunks
SCALE = HD ** -0.5
ESHIFT = -3.0   # exp(x*scale + ESHIFT): cancels in softmax, keeps e < fp8 max
WSCALE = 8.0    # weights pre-scaled into fp8; epilogues multiply by 1/WSCALE
ADIV = 64.0     # attention outputs stored as raw/ADIV in fp8; rsum carries xADIV
RSQRT_MAGIC = 0x5F3759DF

_NC_CACHE = {}


class Ctx:
    pass


def _consts_early(nc, c, const, nw_d, nb_d, qb_d, pb_d):
    # bias vectors load as single-descriptor ROWS (a (128,1)-column DMA costs
    # ~1.4us of queue time; a contiguous row is free) - PE transposes turn
    # them into per-partition columns right after the identity exists
    c.nwrow = const.tile([1, C], F32, tag="nwrow")
    c.nbrow = const.tile([1, C], F32, tag="nbrow")
    c.pbrow = const.tile([1, C], F32, tag="pbrow")
    c.qbrow = const.tile([1, 3 * C], F32, tag="qbrow")
    nc.sync.dma_start(c.nwrow, nw_d.ap()[None, :])
    nc.sync.dma_start(c.nbrow, nb_d.ap()[None, :])
    nc.sync.dma_start(c.pbrow, pb_d.ap()[None, :])
    nc.sync.dma_start(c.qbrow, qb_d.ap()[None, :])

    # only what groupnorm + the first transposes need; everything else is
    # deferred so it can't block the DVE/gpsimd in-order queues at startup
    c.ident = const.tile([P, P], F32, tag="ident")
    make_identity(nc, c.ident)

    # gmask[kc][ch, g] = 1/16 iff global_channel // 16 == g   (128, 32)
    c.gmask = []
    for kc in range(CK):
        gm = const.tile([P, NG], F32, tag=f"gmask{kc}", name=f"gmask{kc}")
        nc.gpsimd.memset(gm, 1.0 / GS)
        nc.gpsimd.affine_select(
            out=gm, in_=gm, compare_op=OP.is_ge, fill=0.0,
            base=P * kc, channel_multiplier=1, pattern=[[-GS, NG]])
        nc.gpsimd.affine_select(
            out=gm, in_=gm, compare_op=OP.is_ge, fill=0.0,
            base=(GS - 1) - P * kc, channel_multiplier=-1, pattern=[[GS, NG]])
        c.gmask.append(gm)

    # bmask[g, ch] = 1 iff ch // 16 == g  (32, 512)
    c.bmask = const.tile([NG, C], F32, tag="bmask")
    nc.gpsimd.memset(c.bmask, 1.0)
    nc.gpsimd.affine_select(
        out=c.bmask, in_=c.bmask, compare_op=OP.is_ge, fill=0.0,
        base=0, channel_multiplier=-GS, pattern=[[1, C]])
    nc.gpsimd.affine_select(
        out=c.bmask, in_=c.bmask, compare_op=OP.is_ge, fill=0.0,
        base=GS - 1, channel_multiplier=GS, pattern=[[-1, C]])

    c.magic = const.tile([NG, 1], U32, tag="magic")
    nc.vector.memset(c.magic, RSQRT_MAGIC)
    c.eshift = const.tile([P, 1], F32, tag="eshift")
    nc.vector.memset(c.eshift, ESHIFT)
    c.ones1 = const.tile([1, P], F32, tag="ones1")
    nc.vector.memset(c.ones1, 1.0)


def _consts_late(nc, c, const):
    # sel2[h2, ch] = 1 iff ch // 64 == h2  (2, 128), f32r for full-rate matmul
    sel2s = const.tile([2, P], F32, tag="sel2s")
    nc.gpsimd.memset(sel2s, 1.0)
    nc.gpsimd.affine_select(
        out=sel2s, in_=sel2s, compare_op=OP.is_ge, fill=0.0,
        base=0, channel_multiplier=-HD, pattern=[[1, P]])
    nc.gpsimd.affine_select(
        out=sel2s, in_=sel2s, compare_op=OP.is_ge, fill=0.0,
        base=HD - 1, channel_multiplier=HD, pattern=[[-1, P]])
    c.sel2 = const.tile([2, P], F32R, tag="sel2")
    nc.vector.tensor_copy(out=c.sel2, in_=sel2s)


def _emit(nc, tc, pools, x_d, out_d, nw_d, nb_d, qw_d, qb_d, pw_d, pb_d):
    (const, stage, xp, hp_, qkp, vp, ep, attp, op_, sm, csp, ps, ps2) = pools

    qi = Ctx()
    qi.steady_i = 0

    def next_q():
        e = [nc.sync, nc.gpsimd][qi.steady_i % 2]
        qi.steady_i += 1
        return e

    c = Ctx()

    x_r = x_d.ap().rearrange("b (kc p) h w -> b kc p (h w)", p=P)
    o_r = out_d.ap().rearrange("b (kc p) h w -> b kc p (h w)", p=P)

    S = [Ctx() for _ in range(BPC)]
    for st_ in S:
        st_.x = [None] * CK

    def emit_x_load(s, engines, split=True):
        # split each (128, 1024) tile into two partition halves spread over
        # the queues so every kc chunk completes early
        st_ = S[s]
        for kc in range(CK):
            xt = xp.tile([P, L], F32, tag=f"x{kc}", name=f"x{kc}_{s}")
            if split:
                for half in range(2):
                    sl = slice(half * 64, (half + 1) * 64)
                    engines[(2 * kc + half) % len(engines)].dma_start(
                        xt[sl, :], x_r[s, kc][sl, :])
            else:
                engines[kc % len(engines)].dma_start(xt, x_r[s, kc])
            st_.x[kc] = xt

    def emit_bias_cols():
        # transpose the bias rows into per-partition columns in one PSUM
        # pass; broadcast the v bias via a K=1 ones-matmul
        bp = ps2.tile([P, 512], F32, tag="p2", name="bias_ps")
        one = c.ones1[0:1, 0:1]   # 1x1 identity for single-row transposes
        for kc in range(CK):
            nc.tensor.transpose(bp[:, kc:kc + 1],
                                c.nwrow[:, kc * P:(kc + 1) * P], one)
            nc.tensor.transpose(bp[:, 4 + kc:5 + kc],
                                c.nbrow[:, kc * P:(kc + 1) * P], one)
            nc.tensor.transpose(bp[:, 16 + kc:17 + kc],
                                c.pbrow[:, kc * P:(kc + 1) * P], one)
        for oc in range(8):
            nc.tensor.transpose(bp[:, 8 + oc:9 + oc],
                                c.qbrow[:, oc * P:(oc + 1) * P], one)
        bias_cols = const.tile([P, 20], F32, tag="bias_cols")
        nc.vector.tensor_copy(out=bias_cols, in_=bp[:, 0:20])
        c.nwall = bias_cols[:, 0:4]
        c.nball = bias_cols[:, 4:8]
        c.qb = [bias_cols[:, 8 + oc:9 + oc] for oc in range(8)]
        c.pb = [bias_cols[:, 16 + kc:17 + kc] for kc in range(CK)]

    def emit_vb():
        vps = ps2.tile([P, 512], F32, tag="p2", name="vb_ps")
        nc.tensor.matmul(vps, c.ones1, c.qbrow[:, 1024:1536],
                         start=True, stop=True)
        c.vb = const.tile([P, 512], F32, tag="vb")
        nc.vector.tensor_copy(out=c.vb, in_=vps)

    def emit_gn_stats(s, kcs=None):
        st_ = S[s]
        if not hasattr(st_, "stat2"):
            st_.stat2 = [None] * CK
        for kc in (range(CK) if kcs is None else kcs):
            xt = st_.x[kc]
            bst = sm.tile([P, 2, 6], F32, tag="bst", name="bst")
            nc.vector.bn_stats(out=bst[:, 0, :], in_=xt[:, 0:512])
            nc.vector.bn_stats(out=bst[:, 1, :], in_=xt[:, 512:1024])
            mv = sm.tile([P, 2], F32, tag="mv", name="mv")
            nc.vector.bn_aggr(out=mv, in_=bst)
            st2 = sm.tile([P, 2], F32, tag="st2", name="st2")
            nc.vector.tensor_copy(out=st2[:, 0:1], in_=mv[:, 0:1])
            nc.vector.tensor_tensor(st2[:, 1:2], mv[:, 0:1], mv[:, 0:1], OP.mult)
            nc.vector.tensor_tensor(st2[:, 1:2], st2[:, 1:2], mv[:, 1:2], OP.add)
            st_.stat2[kc] = st2

    def emit_rsqrt(dst, var):
        # dst = (var + EPS) ** -0.5 entirely on DVE: shift-subtract seed +
        # two Newton-Raphson passes (~1e-5 rel) - keeps ACT's exp table hot
        vv = sm.tile([NG, 1], F32, tag="vv", name="vv")
        y = sm.tile([NG, 1], F32, tag="y", name="y")
        t1 = sm.tile([NG, 1], F32, tag="t1", name="t1")
        nc.vector.tensor_scalar(vv, var, EPS, None, op0=OP.add)
        nc.vector.tensor_scalar(y.bitcast(U32), vv.bitcast(U32), 1, None,
                                op0=OP.logical_shift_right)
        nc.vector.tensor_tensor(y.bitcast(U32), c.magic, y.bitcast(U32),
                                OP.subtract)
        for _ in range(2):
            nc.vector.tensor_tensor(t1, vv, y, OP.mult)
            nc.vector.tensor_tensor(t1, t1, y, OP.mult)
            nc.vector.tensor_scalar(t1, t1, -0.5, 1.5, op0=OP.mult, op1=OP.add)
            nc.vector.tensor_tensor(y, y, t1, OP.mult)
        nc.vector.tensor_copy(out=dst, in_=y)

    def emit_gn_reduce(s):
        st_ = S[s]
        gps = ps2.tile([P, 512], F32, tag="p2", name="gn_ps")
        for kc in range(CK):
            nc.tensor.matmul(gps[0:NG, 0:2], c.gmask[kc], st_.stat2[kc],
                             start=(kc == 0), stop=(kc == CK - 1))
        gst = sm.tile([NG, 2], F32, tag="gst", name="gst")
        gsb = sm.tile([NG, 2], F32, tag="gsb", name="gsb")
        gtmp = sm.tile([NG, 1], F32, tag="gtmp", name="gtmp")
        nc.vector.tensor_copy(out=gsb, in_=gps[0:NG, 0:2])
        nc.vector.tensor_tensor(gtmp, gsb[:, 0:1], gsb[:, 0:1], OP.mult)
        nc.vector.tensor_tensor(gtmp, gsb[:, 1:2], gtmp, OP.subtract)  # var
        emit_rsqrt(gst[:, 1:2], gtmp)                                  # rstd
        nc.vector.tensor_copy(out=gst[:, 0:1], in_=gsb[:, 0:1])       # gmean
        chps = ps2.tile([P, 512], F32, tag="p2", name="gn_ps2")
        for kc in range(CK):
            nc.tensor.matmul(chps[:, kc * 2: kc * 2 + 2],
                             c.bmask[:, kc * P:(kc + 1) * P], gst,
                             start=True, stop=True)
        # batched per-channel scale/shift: 3 strided DVE ops, not 12 smalls
        ch_v = chps.rearrange("p (kc two) -> p kc two", two=2)
        st_.AB = sm.tile([P, 2, CK, 1], F32, tag="AB", name=f"AB_{s}")
        A, Bv = st_.AB[:, 0], st_.AB[:, 1]
        nc.vector.tensor_tensor(A, ch_v[:, 0:CK, 1:2],
                                c.nwall[:, :, None], OP.mult)
        nc.vector.tensor_tensor(Bv, ch_v[:, 0:CK, 0:1], A, OP.mult)
        nc.vector.tensor_tensor(Bv, c.nball[:, :, None], Bv, OP.subtract)
        st_.h = [None, None]   # fp8 channel-chunk-pair tiles (128, 2, 1024)
        st_.qkT = [None] * 8
        st_.v = [None] * (LK // 2)
        st_.att = [None, None]  # fp8 pair tiles (128, 2, 1024), t = hp % 2

    def emit_gn_h(s, kc):
        st_ = S[s]
        kcp, t = kc // 2, kc % 2
        if st_.h[kcp] is None:
            st_.h[kcp] = hp_.tile([P, 2, L], F8, tag=f"h{kcp}",
                                  name=f"h{kcp}_{s}")
        # alternate DVE/GpSimd so the four h writes take ~2 writes of wall
        # time on the first-exp critical path
        eng = nc.vector if kc % 2 else nc.gpsimd
        with nc.allow_low_precision(reason="fp8 activations"):
            eng.tensor_scalar(st_.h[kcp][:, t, :], st_.x[kc],
                              st_.AB[:, 0, kc], st_.AB[:, 1, kc],
                              op0=OP.mult, op1=OP.add)

    def emit_gn_apply(s):
        emit_gn_reduce(s)
        for kc in range(CK):
            emit_gn_h(s, kc)

    qw_r4 = qw_d.ap().rearrange("(oc p) ch -> oc p ch", p=P)
    pw_r4 = pw_d.ap().rearrange("(oc p) ch -> oc p ch", p=P)
    # wT: (128, kcp, t, col) fp8, pre-scaled by WSCALE.  col < 1536 for qkv.
    c.wT = const.tile([P, 2, 2, 3 * C], F8, tag="wT", name="wT")
    c.pT = const.tile([P, 2, 2, C], F8, tag="pT", name="pT")

    def emit_tr_stage(oc, eng=None):
        src_r = qw_r4[oc] if oc < 12 else pw_r4[oc - 12]
        ws = stage.tile([P, C], F32, tag="wstage", name="wstage")
        (eng or next_q()).dma_start(ws, src_r)
        return ws

    def emit_tr_unit(oc, ws, on_act=False):
        col = (oc if oc < 12 else oc - 12) * P
        pt = ps2.tile([P, 512], F32, tag="p2", name="tr_ps")
        for kc in range(CK):
            nc.tensor.transpose(pt[:, kc * P:(kc + 1) * P],
                                ws[:, kc * P:(kc + 1) * P], c.ident)
        dst = c.wT if oc < 12 else c.pT
        dst_ap = dst[:, :, :, col:col + P]
        src_ap = pt.rearrange("p (a b i) -> p a b i", a=2, i=P)
        with nc.allow_low_precision(reason="fp8 weights, x8 prescaled"):
            if on_act:
                # startup only: ACT is idle pre-attention, and this keeps the
                # x8-cast copies out of the groupnorm-critical DVE queue
                nc.scalar.activation(dst_ap, src_ap, AF.Copy, scale=WSCALE)
            else:
                nc.vector.tensor_scalar(dst_ap, src_ap, WSCALE, None,
                                        op0=OP.mult)

    def emit_qkv_unit(s, oc, li):
        st_ = S[s]
        if st_.qkT[oc] is None:
            st_.qkT[oc] = qkp.tile([P, L], BF16, tag=f"qk{oc}", name=f"qk{oc}_{s}")
        dst = st_.qkT[oc]
        pt = ps2.tile([P, 512], F32, tag="p2", name="qkv_ps")
        for kcp in range(2):
            nc.tensor.matmul(pt,
                             c.wT[:, kcp, :, oc * P:(oc + 1) * P],
                             st_.h[kcp][:, :, li * 512:(li + 1) * 512],
                             start=(kcp == 0), stop=(kcp == 1),
                             perf_mode=PM.DoubleRow)
        nc.vector.tensor_scalar(dst[:, li * 512:(li + 1) * 512],
                                pt, 1.0 / WSCALE, c.qb[oc],
                                op0=OP.mult, op1=OP.add)

    def emit_qkv_qk(s, hp):
        for oc in (hp, 4 + hp):
            for li in range(2):
                emit_qkv_unit(s, oc, li)

    def emit_v(s, lcs):
        # v pair tiles: (128 jpix, 2 chunk-parity, 8 heads, 64+1+3pad) fp8;
        # head pitch 68 keeps DoubleRow ldweights panels 4-byte aligned
        st_ = S[s]
        for lc in lcs:
            lcp, t = lc // 2, lc % 2
            pt = ps2.tile([P, 512], F32, tag="p2", name="v_ps")
            for kcp in range(2):
                nc.tensor.matmul(pt,
                                 st_.h[kcp][:, :, lc * P:(lc + 1) * P],
                                 c.wT[:, kcp, :, 1024:1536],
                                 start=(kcp == 0), stop=(kcp == 1),
                                 perf_mode=PM.DoubleRow)
            if st_.v[lcp] is None:
                vt = vp.tile([P, 2, NH, HD + 4], F8, tag=f"v{lcp}",
                             name=f"v{lcp}_{s}")
                nc.vector.memset(vt[:, :, :, HD:HD + 1], 1.0)
                nc.vector.memset(vt[:, :, :, HD + 1:HD + 4], 0.0)
                st_.v[lcp] = vt
            vt = st_.v[lcp]
            with nc.allow_low_precision(reason="fp8 attention values"):
                nc.vector.scalar_tensor_tensor(
                    out=vt[:, t, :, 0:HD],
                    in0=pt.rearrange("p (h d) -> p h d", d=HD),
                    scalar=1.0 / WSCALE,
                    in1=c.vb.rearrange("p (h d) -> p h d", d=HD),
                    op0=OP.mult, op1=OP.add)

    fill_q = []    # units for the sample-0 attention window (drains first)
    fill_q2 = []   # reserve units held back for the sample-1 window

    def pop_fill(n=1):
        for _ in range(n):
            if fill_q:
                fill_q.pop(0)()
            elif fill_q2:
                fill_q2.pop(0)()

    def make_norm2(s, hp, li, rsum):
        st_ = S[s]

        def norm2():
            rb2 = ps2.tile([P, 512], F32, tag="p2", name="rb2_ps")
            nc.tensor.matmul(rb2, c.sel2, rsum[:, li * 512:(li + 1) * 512],
                             start=True, stop=True)
            kcp, t = hp // 2, hp % 2
            sl = st_.att[kcp][:, t, li * 512:(li + 1) * 512]
            with nc.allow_low_precision(reason="fp8 attention probs"):
                nc.vector.tensor_tensor(sl, sl, rb2, OP.mult)
        return norm2

    carry = Ctx()
    carry.stile = None

    def s_mms_for(s2, hp2, ic, jc):
        st2 = S[s2]
        kT, qT = st2.qkT[4 + hp2], st2.qkT[hp2]
        stile = ps.tile([P, 1024], F32, tag="s", name=f"s_{hp2}_{ic}_{jc}")
        for h2 in range(2):
            nc.tensor.matmul(
                stile[:, h2 * 512:(h2 + 1) * 512],
                kT[h2 * HD:(h2 + 1) * HD, jc * P:(jc + 1) * P],
                qT[h2 * HD:(h2 + 1) * HD, ic * 512:(ic + 1) * 512],
                start=True, stop=True)
        return stile

    def emit_pair(s, hp, tail_units=None):
        st_ = S[s]
        kcp_a, t_a = hp // 2, hp % 2
        if st_.att[kcp_a] is None:
            st_.att[kcp_a] = attp.tile([P, 2, L], F8, tag=f"att{kcp_a}",
                                       name=f"att{kcp_a}_{s}")
        csum = csp.tile([2, L], F32, tag="csum", name=f"csum_{s}_{hp}")

        def s_mms(ic, jc):
            return s_mms_for(s, hp, ic, jc)

        def next_block(ic):
            # the block whose first S-matmuls we prefetch during this
            # block's last step, so the exp stream never waits at a boundary
            if ic == 0:
                return (s, hp, 1)
            if hp < 3 and S[s].qkT[hp + 5] is not None:
                return (s, hp + 1, 0)
            return None

        for ic in range(2):
            av = ps.tile([P, 1024], F32, tag="s", name=f"av_{hp}_{ic}")
            if carry.stile is not None:
                stile, carry.stile = carry.stile, None
            else:
                stile = s_mms(ic, 0)
            for jcp in range(LK // 2):
                e_t = ep.tile([P, 2, 1024], F8, tag="e", name="e_t")
                for t in range(2):
                    jc = 2 * jcp + t
                    nc.scalar.activation(e_t[:, t, :], stile, AF.Exp,
                                         scale=SCALE, bias=c.eshift)
                    # emit next S ahead of this AV so the PE stream runs one
                    # step ahead of ScalarE; soak the PE with filler units
                    if jc + 1 < LK:
                        stile = s_mms(ic, jc + 1)
                    else:
                        nb = next_block(ic)
                        if nb is not None:
                            carry.stile = s_mms_for(nb[0], nb[1], nb[2], 0)
                    pop_fill()
                for h2 in range(2):
                    nc.tensor.matmul(
                        av[0:HD + 2, h2 * 512:(h2 + 1) * 512],
                        st_.v[jcp][:, :, 2 * hp + h2, 0:HD + 2],
                        e_t[:, :, h2 * 512:(h2 + 1) * 512],
                        start=(jcp == 0), stop=(jcp == LK // 2 - 1),
                        perf_mode=PM.DoubleRow)
            for h2 in range(2):
                with nc.allow_low_precision(reason="fp8, /64 range guard"):
                    nc.vector.tensor_scalar(
                        st_.att[kcp_a][h2 * HD:(h2 + 1) * HD, t_a,
                                       ic * 512:(ic + 1) * 512],
                        av[0:HD, h2 * 512:(h2 + 1) * 512],
                        1.0 / ADIV, None, op0=OP.mult)
                cstage = sm.tile([1, 512], F32, tag="cstage", name="cstage")
                nc.vector.tensor_copy(
                    out=cstage, in_=av[HD:HD + 1, h2 * 512:(h2 + 1) * 512])
                # NOTE: never issue DMAs from the scalar engine - any HWDGE
                # work on ACT slows every activation ~20% for the whole run
                nc.sync.dma_start(csum[h2:h2 + 1, ic * 512:(ic + 1) * 512], cstage)
            # per-half denominators: approx reciprocal on DVE, x64 rounding
            # copy (f32r for the selector matmul) on gpsimd (DVE for the
            # tail-critical second sample), then the normalize unit queues
            rscr = csp.tile([2, 512], F32, tag="rscr", name=f"rscr_{s}_{hp}_{ic}")
            rsum = csp.tile([2, L], F32R, tag="rsum", name=f"rsum_{s}_{hp}",
                            bufs=2) if ic == 0 else st_.rsum_cur
            st_.rsum_cur = rsum
            with nc.allow_low_precision(reason="softmax denominators"):
                nc.vector.reciprocal_approx_fast(
                    out=rscr, in_=csum[:, ic * 512:(ic + 1) * 512])
            nc.vector.tensor_scalar(rsum[:, ic * 512:(ic + 1) * 512], rscr,
                                    ADIV, None, op0=OP.mult)
            fill_q.insert(min(len(fill_q), 6), make_norm2(s, hp, ic, rsum))
            if tail_units and tail_units.get(ic):
                pos = min(len(fill_q), 7)
                for u in tail_units[ic]:
                    fill_q.insert(pos, u)
                    pos += 1

    def emit_proj_unit(s, oc, li):
        st_ = S[s]
        pt = ps2.tile([P, 512], F32, tag="p2", name="proj_ps")
        for kcp in range(2):
            nc.tensor.matmul(pt,
                             c.pT[:, kcp, :, oc * P:(oc + 1) * P],
                             st_.att[kcp][:, :, li * 512:(li + 1) * 512],
                             start=(kcp == 0), stop=(kcp == 1),
                             perf_mode=PM.DoubleRow)
        ot = op_.tile([P, 512], F32, tag="ot", name="ot")
        nc.vector.tensor_scalar(ot, pt, 1.0 / WSCALE, c.pb[oc],
                                op0=OP.mult, op1=OP.add)
        nc.gpsimd.tensor_tensor(ot, ot, st_.x[oc][:, li * 512:(li + 1) * 512],
                                OP.add)
        for q in range(2):
            sl = slice(li * 512 + q * 256, li * 512 + (q + 1) * 256)
            next_q().dma_start(o_r[s, oc][:, sl], ot[:, q * 256:(q + 1) * 256])

    # ---------------- schedule ----------------
    # startup: bias rows (single-descriptor) then x s0 exclusively on the two
    # fast HWDGE queues; ALL weight stages ride the gpsimd SWDGE queue (its
    # engine builds the mask consts first, then fires the triggers); x s1
    # trails x s0.  Sample-1 groupnorm runs as filler units so it can't
    # steal DVE time from the sample-0 critical chain.
    _consts_early(nc, c, const, nw_d, nb_d, qb_d, pb_d)
    emit_x_load(0, [nc.sync, nc.scalar], split=False)
    ws_first = {}
    for oc in (0, 4, 8, 9, 10, 11):
        ws_first[oc] = emit_tr_stage(oc, nc.gpsimd)
    emit_x_load(1, [nc.sync, nc.scalar])
    emit_gn_stats(0)
    emit_bias_cols()
    for oc in (0, 4):
        emit_tr_unit(oc, ws_first[oc])
    emit_gn_apply(0)
    emit_qkv_qk(0, 0)         # pair(0,0) q/k: its DVE epilogues gate the
    for oc in (8, 9, 10, 11):
        emit_tr_unit(oc, ws_first[oc])
    emit_vb()
    _consts_late(nc, c, const)
    emit_v(0, [0, 1, 2, 3])   # first S-matmuls gate on these

    # everything else becomes filler units popped per attention step; the
    # fill_q order encodes the just-in-time deadlines.  Units that may wait
    # until the sample-1 attention window go to the fill_q2 reserve, so the
    # PE stays fed (and at high p-state) through BOTH windows.
    for lc in (4, 5):
        fill_q.append(lambda lc=lc: emit_v(0, [lc]))
    for kc in (0, 1):
        fill_q.append(lambda kc=kc: emit_gn_stats(1, [kc]))
    for lc in (6, 7):
        fill_q.append(lambda lc=lc: emit_v(0, [lc]))
    for kc in (2, 3):
        fill_q.append(lambda kc=kc: emit_gn_stats(1, [kc]))
    for oc_t, oc_a, oc_b in ((1, 1, 5),):
        fill_q.append(lambda oc=oc_t: emit_tr_unit(oc, emit_tr_stage(oc)))
        fill_q.append(lambda oc=oc_t: emit_tr_unit(oc + 4, emit_tr_stage(oc + 4)))
        for li in range(2):
            fill_q.append(lambda oc=oc_a, li=li: emit_qkv_unit(0, oc, li))
        for li in range(2):
            fill_q.append(lambda oc=oc_b, li=li: emit_qkv_unit(0, oc, li))
    fill_q.append(lambda: emit_gn_reduce(1))
    for kc in range(CK):
        fill_q.append(lambda kc=kc: emit_gn_h(1, kc))
    for oc_t, oc_a, oc_b in ((2, 2, 6), (3, 3, 7)):
        fill_q.append(lambda oc=oc_t: emit_tr_unit(oc, emit_tr_stage(oc)))
        fill_q.append(lambda oc=oc_t: emit_tr_unit(oc + 4, emit_tr_stage(oc + 4)))
        for li in range(2):
            fill_q.append(lambda oc=oc_a, li=li: emit_qkv_unit(0, oc, li))
        for li in range(2):
            fill_q.append(lambda oc=oc_b, li=li: emit_qkv_unit(0, oc, li))
    for oc in (12, 13, 14, 15):       # proj weights, needed from pair(1,0)
        fill_q.append(lambda oc=oc: emit_tr_unit(oc, emit_tr_stage(oc)))
    for oc in (0, 4):                 # pair(1,0)'s q/k - before its S mms
        for li in range(2):
            fill_q.append(lambda oc=oc, li=li: emit_qkv_unit(1, oc, li))
    for lc in range(LK):              # all v(1) - consumed from pair(1,0) on
        fill_q.append(lambda lc=lc: emit_v(1, [lc]))
    for oc in (1, 5, 2, 6, 3, 7):     # later pairs' q/k ride the reserve
        for li in range(2):
            fill_q2.append(lambda oc=oc, li=li: emit_qkv_unit(1, oc, li))

    for hp in range(3):
        emit_pair(0, hp)
    emit_pair(0, 3, tail_units={
        0: [lambda oc=oc: emit_proj_unit(0, oc, 0) for oc in range(CK)],
        1: [lambda oc=oc: emit_proj_unit(0, oc, 1) for oc in range(CK)],
    })
    while fill_q:   # anything pair(1,0) needs that hasn't popped yet
        pop_fill()

    for hp in range(3):
        emit_pair(1, hp)
    emit_pair(1, 3, tail_units={
        0: [lambda oc=oc: emit_proj_unit(1, oc, 0) for oc in range(CK)],
        1: [lambda oc=oc: emit_proj_unit(1, oc, 1) for oc in range(CK)],
    })
    while fill_q or fill_q2:
        pop_fill()


def _build():
    if "nc" in _NC_CACHE:
        return _NC_CACHE["nc"]
    nc = bacc.Bacc("TRN2", target_bir_lowering=False, debug=False)
    x_d = nc.dram_tensor("x", (BPC, C, H, W), F32, kind="ExternalInput")
    nw_d = nc.dram_tensor("norm_w", (C,), F32, kind="ExternalInput")
    nb_d = nc.dram_tensor("norm_b", (C,), F32, kind="ExternalInput")
    qw_d = nc.dram_tensor("qkv_w", (3 * C, C), F32, kind="ExternalInput")
    qb_d = nc.dram_tensor("qkv_b", (3 * C,), F32, kind="ExternalInput")
    pw_d = nc.dram_tensor("proj_w", (C, C), F32, kind="ExternalInput")
    pb_d = nc.dram_tensor("proj_b", (C,), F32, kind="ExternalInput")
    out_d = nc.dram_tensor("out", (BPC, C, H, W), F32, kind="ExternalOutput")
    with tile.TileContext(nc) as tc:
        with (
            tc.tile_pool(name="const", bufs=1) as const,
            tc.tile_pool(name="stage", bufs=6) as stage,
            tc.tile_pool(name="xp", bufs=2) as xp,
            tc.tile_pool(name="hp", bufs=2) as hp_,
            tc.tile_pool(name="qkp", bufs=2) as qkp,
            tc.tile_pool(name="vp", bufs=2) as vp,
            tc.tile_pool(name="ep", bufs=3) as ep,
            tc.tile_pool(name="attp", bufs=2) as attp,
            tc.tile_pool(name="op", bufs=2) as op_,
            tc.tile_pool(name="sm", bufs=1) as sm,
            tc.tile_pool(name="csp", bufs=2) as csp,
            tc.tile_pool(name="ps", bufs=3, space="PSUM") as ps,
            tc.tile_pool(name="ps2", bufs=2, space="PSUM") as ps2,
        ):
            pools = (const, stage, xp, hp_, qkp, vp, ep, attp, op_, sm, csp, ps, ps2)
            _emit(nc, tc, pools, x_d, out_d, nw_d, nb_d, qw_d, qb_d, pw_d, pb_d)
    nc.compile()
    _NC_CACHE["nc"] = nc
    return nc


def kernel(x, norm_w, norm_b, qkv_w, qkv_b, proj_w, proj_b):
    x = np.ascontiguousarray(x, dtype=np.float32)
    args = {
        "norm_w": np.ascontiguousarray(norm_w, np.float32),
        "norm_b": np.ascontiguousarray(norm_b, np.float32),
        "qkv_w": np.ascontiguousarray(qkv_w, np.float32),
        "qkv_b": np.ascontiguousarray(qkv_b, np.float32),
        "proj_w": np.ascontiguousarray(proj_w, np.float32),
        "proj_b": np.ascontiguousarray(proj_b, np.float32),
    }
    nc = _build()
    in_maps = [dict(args, x=x[i * BPC:(i + 1) * BPC]) for i in range(N_CORES)]
    res = run_bass_kernel_spmd(nc, in_maps, core_ids=list(range(N_CORES)))
    return np.concatenate([r["out"] for r in res.results], axis=0)
